# revision 61
# baseline (speedup 1.0000x reference)
"""Causal self-attention (B=4, T=2048, C=1024, NH=16) on 8 trn2 NeuronCores.

Sharding: core = (head_group hg in {0,1}) x (batch b in {0..3}).
Each core computes qkv projection + attention + partial output projection for
its 8 heads of its batch; host sums the two head-group partials per batch and
adds the output bias.

v3 = v2 + fp8 DoubleRow "3-slot split" for the qkv and output projections.
  - A matmul instruction costs out_free_size x cycles_per_row; fp8e4/e5 with
    perf_mode=DoubleRow runs at 0.5 cycles/row and contracts TWO 128-row
    k-tiles per instruction (lhsT [128,2,M], rhs [128,2,N]).  Writing
    X = Xh + Xl and W = Wh + Wl (each an e4m3 pair: hi = e4(x), lo =
    e4(x - hi), together ~9 mantissa bits > bf16's 8), the product
    X@W ~= Xh@Wh + Xh@Wl + Xl@Wh needs 3 slot-products per k-tile = 1.5
    DoubleRow instructions per k-tile pair = 0.75x the bf16 PE cost.
    Measured end-to-end rel-err 4.3e-3 vs bf16's 5.6e-3 (the e4m3 pair is
    slightly MORE precise than bf16).
  - Scale scheme keeps every fp8 operand in e4m3 normal range with all
    compensation factors exact powers of two: x' = 16x, W' = 64Wqkv =>
    PSUM q/k/v are 1024x; bias ships 1024x; scores are 2^20 x so the exp
    scale is 0.125*2^-20; the AV ones-column is 64.0 so o = acc*dinv comes
    out 16x; Wproj' = 64Wproj => y partials are 1024x, divided on the host.
  - qkv/V weights and x ship as host-packed hi/lo e4m3 pairs in DR-friendly
    row order (slot pairs contiguous), so SBUF tiles load with plain DMAs:
    same total bytes as the bf16 v2 (hi+lo = 2 bytes/elem).
  - o (= 16x true o, bf16) transposes to oT exactly as v2 (one
    dma_start_transpose per 128-query chunk), then splits on DVE into
    oT_hi = e4(oT), oT_lo = e4(oT - oT_hi) for the deferred DR projection.
  - scores and AV stay bf16: the score contraction is only 64 (no k-tile
    pair to fuse) and AV's pt residual would need a second exp pass.

Attention core (unchanged from v2):
  - q/k computed transposed (head_size on partitions); V in natural
    [token, feat] layout; V bias folded into the host-side output bias.
  - S^T = K @ Q^T per (head, 128-key block, 128-query chunk); 8 causal
    blocks packed in one [128, 1024] PSUM tile so one ScalarE Exp covers
    them.  Causal mask = one 0/1 multiply per diagonal block on GpSimd.
  - AV runs query-on-partitions: acc[q, 0:65] += pt_slice^T @ [V | 64];
    the softmax denominator arrives as a per-partition scalar -> DVE
    reciprocal + tensor_scalar_mul.
  - output projections deferred to the end of the program as PE fill for
    the ACT-bound late tiles.
  - a few throwaway warmup matmuls burn the PE p-state ramp.
Cost-model span: see test.py (v2 was 203588 ns/core; PE busy was 97.5%).
"""

import sys

sys.path.insert(0, "/opt/trn_rl_repo")

import numpy as np

import concourse.bacc as bacc
import concourse.bass as bass
import concourse.mybir as mybir
from concourse.bass_utils import run_bass_kernel_spmd
from concourse.tile import TileContext

B, T, C, NH = 4, 2048, 1024, 16
HS = C // NH          # 64
HGF = 512             # features per head group (8 heads x 64)
QT = 256              # query tile (S stage)
NKT = T // 128        # 16 key tiles
F32 = mybir.dt.float32
BF16 = mybir.dt.bfloat16
E4 = mybir.dt.float8e4
DR = mybir.MatmulPerfMode.DoubleRow
Exp = mybir.ActivationFunctionType.Exp

SA = 16.0             # x scale
SB = 64.0             # weight scale
EXP_SCALE = 0.125 / float(SA * SA * SB * SB)   # 0.125 * 2^-20
ONES_VAL = 64.0       # AV denominator column value => o = 16x true o
OUT_DIV = SA * SB     # host divides y partials by 1024
# query chunks j >= PT8J run attention weights in e4m3: exp emits pt8 =
# exp(s - PT8_SHIFT) (max scaled score is 9.04 -> pt8 <= e^5.04 = 155 < 448;
# rows have >= 513 keys so a row of all-zero pt8 is impossible), V ships as
# a 16x e4m3 hi/lo pair, and AV runs DoubleRow over key-tile pairs at half
# the bf16 PE cost.  The pt8 denominator column (1.0) normalizes with the
# same quantized weights, cancelling most of the quantization error.
PT8J = 2
PT8_SHIFT = -4.0
RG = 1                # S-groups fused per exp instruction (1 = double-buffered)


def build_kernel():
    nc = bacc.Bacc(None, target_bir_lowering=False)
    # packed x: row = 128*pr + p, col = 1024*n + 512*two + t
    xh = nc.dram_tensor("xh", (512, 4096), E4, kind="ExternalInput")
    xl = nc.dram_tensor("xl", (512, 4096), E4, kind="ExternalInput")
    # packed q/k weights: row = 128*m + p (m 0..3 q, 4..7 k),
    # col = 256*pr + 128*two + c
    wqkh = nc.dram_tensor("wqkh", (1024, 1024), E4, kind="ExternalInput")
    wqkl = nc.dram_tensor("wqkl", (1024, 1024), E4, kind="ExternalInput")
    # packed v weights (moving layout): row = 128*pr + p, col = 512*two + c
    wvh = nc.dram_tensor("wvh", (512, 1024), E4, kind="ExternalInput")
    wvl = nc.dram_tensor("wvl", (512, 1024), E4, kind="ExternalInput")
    bqk = nc.dram_tensor("bqk", (128, 8), F32, kind="ExternalInput")
    # packed proj weights: row = 128*g + p, col = 1024*two + c
    wph = nc.dram_tensor("wph", (256, 2048), E4, kind="ExternalInput")
    wpl = nc.dram_tensor("wpl", (256, 2048), E4, kind="ExternalInput")
    mask01 = nc.dram_tensor("mask01", (128, 128), BF16, kind="ExternalInput")
    y = nc.dram_tensor("y", (T, C), F32, kind="ExternalOutput")

    with TileContext(nc) as tc:
        with (
            tc.tile_pool(name="outer", bufs=1) as outer,
            tc.tile_pool(name="work", bufs=1) as work,
            tc.tile_pool(name="psum", bufs=1, space="PSUM") as psum,
        ):
            # ---- PE p-state warmup ----
            warm = outer.tile([128, 512], BF16, name="warm")
            nc.vector.memset(warm, 0.0)
            for wi in range(5):
                wtag, wbufs = ("py", 1) if wi % 2 == 0 else ("mm", 2)
                wps = psum.tile([128, 512], F32, tag=wtag, bufs=wbufs,
                                name=f"warm{wi}")
                nc.tensor.matmul(wps, warm[:, 0:128], warm,
                                 start=True, stop=True)

            bias_all = outer.tile([128, 8], F32, name="bias_all")
            mask_b = outer.tile([128, 128], BF16, name="mask_b")
            # q/k weight m-tiles [p, pr, two, c]; q part (m<4) loads first
            # on the idle scalar queue so the first qkv chain isn't blocked
            wqk_h = [outer.tile([128, 4, 2, 128], E4, name=f"wqh{m}")
                     for m in range(8)]
            wqk_l = [outer.tile([128, 4, 2, 128], E4, name=f"wql{m}")
                     for m in range(8)]
            # pair-priority order: the first S block needs q m0 AND k m4.
            # Only pairs 0/1 ride the scalar queue: its DMA issues occupy
            # the ACT sequencer ~500ns each and would otherwise queue-block
            # the first exp until ~8us.  Pairs 2/3 load later via sync /
            # gpsimd (emitted in the chunk-0 extras below).
            for m in (0, 4, 1, 5):
                nc.scalar.dma_start(
                    wqk_h[m], wqkh[128 * m:128 * m + 128, :].rearrange(
                        "p (pr two c) -> p pr two c", two=2, c=128))
                nc.scalar.dma_start(
                    wqk_l[m], wqkl[128 * m:128 * m + 128, :].rearrange(
                        "p (pr two c) -> p pr two c", two=2, c=128))
            wv_h = [outer.tile([128, 2, 512], E4, name=f"wvh{pr}")
                    for pr in range(4)]
            wv_l = [outer.tile([128, 2, 512], E4, name=f"wvl{pr}")
                    for pr in range(4)]
            wp_h = [outer.tile([128, 2, 1024], E4, name=f"wph{g}")
                    for g in range(2)]
            wp_l = [outer.tile([128, 2, 1024], E4, name=f"wpl{g}")
                    for g in range(2)]

            k_t = [outer.tile([128, T], BF16, name=f"k{i}") for i in range(4)]
            q_sb = [outer.tile([128, T], BF16, name=f"q{i}") for i in range(4)]
            # v_store[i] (bf16, j < PT8J only): [key-tile 128, 8*65]; per head
            # h cols 65h:65h+64 are V features (1024x), col 65h+64 is
            # ONES_VAL (softmax denominator)
            v_store = [outer.tile([128, 8 * 65], BF16, name=f"v{i}")
                       for i in range(2 * PT8J)]
            for i in range(2 * PT8J):
                nc.vector.memset(
                    v_store[i].rearrange("p (g c) -> p g c", c=65)[:, :, 64:65],
                    ONES_VAL,
                )
            # vp_hi/vp_lo[ip] (e4m3, j >= PT8J): key-tile PAIR (2ip, 2ip+1)
            # as DoubleRow slots; per head h cols 65h:65h+64 hold 16x V
            # hi/lo, col 65h+64 is 1.0 in hi / 0.0 in lo (denominator)
            vp_hi = [outer.tile([128, 2, 8 * 65], E4, name=f"vph{ip}")
                     for ip in range(NKT // 2)]
            vp_lo = [outer.tile([128, 2, 8 * 65], E4, name=f"vpl{ip}")
                     for ip in range(NKT // 2)]
            for ip in range(NKT // 2):
                nc.vector.memset(
                    vp_hi[ip].rearrange("p t (g c) -> p t g c", c=65)[:, :, :, 64:65],
                    1.0,
                )
                nc.vector.memset(
                    vp_lo[ip].rearrange("p t (g c) -> p t g c", c=65)[:, :, :, 64:65],
                    0.0,
                )
            nb_sh = outer.tile([128, 1], F32, name="nb_sh")
            nc.vector.memset(nb_sh, PT8_SHIFT)

            # ---- deferred/interleaved output projection machinery ----
            # ready_proj holds (j, s, oT_hi, oT_lo, nn) psy groups whose oT
            # splits are already emitted; emit_psy pops them into the PE
            # stream.  Interleaved groups drain on DVE (ACT is exp-saturated
            # in the late chunks); tail groups split drains as in v2.
            ready_proj = []
            psy_count = [0]

            def emit_psy(j, s, oT_hi, oT_lo, nn, tag, bufs, drain):
                psy = psum.tile([128, 512], F32, tag=tag, bufs=bufs,
                                name=f"py{j}_{s}_{nn}")
                kk = 0
                for g in range(2):
                    for (lt, rt) in ((oT_hi, wp_h[g]), (oT_lo, wp_h[g]),
                                     (oT_hi, wp_l[g])):
                        nc.tensor.matmul(
                            psy,
                            lt[:, 2 * g:2 * g + 2, :],
                            rt[:, :, nn * 512:(nn + 1) * 512],
                            start=(kk == 0), stop=(kk == 5),
                            perf_mode=DR,
                        )
                        kk += 1
                ysb = work.tile([128, 512], F32, tag="ysb", bufs=6,
                                name=f"ys{j}_{s}_{nn}")
                rows = y[j * QT + s * 128:j * QT + (s + 1) * 128, :]
                if drain == "final":
                    # final drain: halves in parallel on two engines and
                    # four DMA queues to shorten the closing chain
                    nc.scalar.activation(
                        ysb[:, 0:256], psy[:, 0:256],
                        mybir.ActivationFunctionType.Copy,
                    )
                    nc.vector.tensor_copy(ysb[:, 256:512], psy[:, 256:512])
                    q0, q1 = ((nc.sync, nc.gpsimd) if psy_count[0] % 2 == 0
                              else (nc.scalar, nc.sync))
                    q0.dma_start(
                        rows[:, nn * 512:nn * 512 + 256], ysb[:, 0:256]
                    )
                    q1.dma_start(
                        rows[:, nn * 512 + 256:nn * 512 + 512],
                        ysb[:, 256:512],
                    )
                else:
                    if drain == "act":
                        nc.scalar.activation(
                            ysb, psy, mybir.ActivationFunctionType.Copy
                        )
                    else:
                        nc.vector.tensor_copy(ysb, psy)
                    yeng = nc.sync if psy_count[0] % 2 == 0 else nc.gpsimd
                    yeng.dma_start(rows[:, nn * 512:(nn + 1) * 512], ysb)
                psy_count[0] += 1

            # pend_av carries the last S-group of a block whose AV matmuls
            # (exp-dependent) haven't been emitted yet; flush_av emits them
            # plus the block's dinv / o-scale ops.
            pend_av = [None]
            # filler machinery: one deferred-proj psy group per attention
            # round gives the in-order PE exp-independent work while ACT
            # drains the single-buffered S tile
            fill_state = {"budget": 0, "rate": 1, "tick": 0}
            fillers = []

            def fill_one():
                if fillers:
                    fillers.pop(0)()
                    return
                fill_state["tick"] += 1
                if (fill_state["budget"] > 0 and ready_proj
                        and fill_state["tick"] % fill_state["rate"] == 0):
                    fill_state["budget"] -= 1
                    emit_psy(*ready_proj.pop(0), tag="py", bufs=1,
                             drain="dve")

            def emit_av(pt_, grp_, acc_, j_, h_):
                for bi, (s, i) in enumerate(grp_):
                    nc.tensor.matmul(
                        acc_[s],
                        pt_[:, bi * 128:(bi + 1) * 128],
                        v_store[i][:, 65 * h_:65 * h_ + 65],
                        start=(i == 0),
                        stop=(i == 2 * j_ + s),
                    )

            post_flush = [None]

            def flush_av():
                if pend_av[0] is None:
                    return
                fn = pend_av[0]
                pend_av[0] = None
                fn()
                if post_flush[0] is not None:
                    fn2 = post_flush[0]
                    post_flush[0] = None
                    fn2()

            oT_parts = {}

            def emit_oT_half(osb, j, s, b0, b1):
                # transpose + split feature blocks [b0, b1) (128 feats each,
                # 2 heads per block) of o tile (j, s); pieces of the last
                # chunk's tiles go out as soon as their heads complete,
                # shortening the closing chain
                if (j, s) not in oT_parts:
                    oT_parts[(j, s)] = (
                        work.tile([128, 4, 128], BF16, tag="ot", bufs=4,
                                  name=f"ot{j}_{s}"),
                        work.tile([128, 4, 128], E4, tag="oth", bufs=16,
                                  name=f"oth{j}_{s}"),
                        work.tile([128, 4, 128], E4, tag="otl", bufs=16,
                                  name=f"otl{j}_{s}"),
                    )
                oT, oT_hi, oT_lo = oT_parts[(j, s)]
                nc.sync.dma_start_transpose(
                    oT[:, b0:b1, :], osb[j][s][:, 128 * b0:128 * b1])
                nc.vector.tensor_copy(oT_hi[:, b0:b1, :], oT[:, b0:b1, :])
                nc.vector.tensor_tensor(
                    oT_lo[:, b0:b1, :], oT[:, b0:b1, :], oT_hi[:, b0:b1, :],
                    mybir.AluOpType.subtract)

            def emit_oT_split(osb, j, s):
                oT = work.tile([128, 4, 128], BF16, tag="ot", bufs=4,
                               name=f"ot{j}_{s}")
                nc.sync.dma_start_transpose(oT, osb[j][s])
                oT_hi = work.tile([128, 4, 128], E4, tag="oth",
                                  bufs=16, name=f"oth{j}_{s}")
                oT_lo = work.tile([128, 4, 128], E4, tag="otl",
                                  bufs=16, name=f"otl{j}_{s}")
                nc.vector.tensor_copy(oT_hi, oT)
                nc.vector.tensor_tensor(
                    oT_lo, oT, oT_hi, mybir.AluOpType.subtract)
                for nn in range(2):
                    ready_proj.append((j, s, oT_hi, oT_lo, nn))

            def emit_attn_pair8(j, p, osb):
                """Pair-merged attention for j >= PT8J: BOTH heads of pair p
                stream through shared S-groups so one exp instruction covers
                up to 8 blocks regardless of the head boundary (160 -> 144
                exps).  acc tiles allocate lazily per head inside the AV
                stream; a head's dinv/o-scale emit the moment its last AV
                lands, releasing the single acc bank for the next head.
                """
                pair = p
                units = []   # (h, s, a, b); b None = s=0 diagonal single
                for h in (2 * p, 2 * p + 1):
                    for a in range(0, 2 * j, 2):
                        units.append((h, 0, a, a + 1))
                    units.append((h, 0, 2 * j, None))
                    for a in range(0, 2 * j + 2, 2):
                        units.append((h, 1, a, a + 1))
                groups, cur, cnt = [], [], 0
                for u in units:
                    w = 1 if u[3] is None else 2
                    if cnt + w > 8:
                        groups.append(cur)
                        cur, cnt = [], 0
                    cur.append((u, cnt))
                    cnt += w
                if cur:
                    groups.append(cur)
                accs = {}

                def av8(pt_, gu, base):
                    for (h, sx, a, b), ca in gu:
                        if h not in accs:
                            acc2 = psum.tile([128, 512], F32, tag="acc",
                                             bufs=1, name=f"acc{j}_{h}")
                            accs[h] = [acc2[:, 256 * s2:256 * s2 + 65]
                                       for s2 in range(2)]
                        acc_ = accs[h]
                        c0 = base + ca * 128
                        st = (a == 0)
                        if b is None:  # s=0 diagonal single: plain fp8 x2
                            for vt, last in ((vp_hi, False), (vp_lo, True)):
                                nc.tensor.matmul(
                                    acc_[sx],
                                    pt_[:, c0:c0 + 128],
                                    vt[a // 2][:, a % 2, 65 * h:65 * h + 65],
                                    start=False,
                                    stop=(sx == 0 and last),
                                )
                        else:
                            ptp = pt_[:, c0:c0 + 256].rearrange(
                                "p (two c) -> p two c", two=2)
                            for vt, last in ((vp_hi, False), (vp_lo, True)):
                                nc.tensor.matmul(
                                    acc_[sx], ptp,
                                    vt[a // 2][:, :, 65 * h:65 * h + 65],
                                    start=(st and vt is vp_hi),
                                    stop=(sx == 1 and b == 2 * j + 1
                                          and last),
                                    perf_mode=DR,
                                )
                        if sx == 1 and b == 2 * j + 1:
                            # head h complete: normalize + release its acc
                            for s2 in range(2):
                                dinv = work.tile([128, 1], F32, tag="dinv",
                                                 bufs=4,
                                                 name=f"di{j}_{h}_{s2}")
                                nc.vector.reciprocal(dinv, acc_[s2][:, 64:65])
                                nc.vector.tensor_scalar_mul(
                                    osb[j][s2][:, 64 * h:64 * h + 64],
                                    acc_[s2][:, 0:64],
                                    dinv,
                                )
                            del accs[h]

                prev_rnd = None
                for r, gu in enumerate(groups):
                    ntot = sum(1 if u[3] is None else 2 for u, _ in gu) * 128
                    sg = psum.tile([128, 1024], F32, tag="big", bufs=2,
                                   name=f"sg{j}_{p}_{r}")
                    for (h, sx, a, b), ca in gu:
                        off = 64 * (h % 2)
                        for bi, i in enumerate([a] if b is None else [a, b]):
                            c0 = (ca + bi) * 128
                            nc.tensor.matmul(
                                sg[:, c0:c0 + 128],
                                k_t[pair][off:off + 64,
                                          i * 128:(i + 1) * 128],
                                q_sb[pair][off:off + 64,
                                           j * QT + sx * 128:
                                           j * QT + sx * 128 + 128],
                                start=True,
                                stop=True,
                            )
                    pt = work.tile([128, 1024], E4, tag="p8", bufs=6,
                                   name=f"p8{j}_{p}_{r}")
                    nc.scalar.activation(
                        pt[:, :ntot], sg[:, :ntot],
                        Exp, scale=EXP_SCALE, bias=nb_sh,
                    )
                    for (h, sx, a, b), ca in gu:
                        if b is None and a == 2 * j:
                            c0 = ca * 128
                        elif b == 2 * j + 1 and sx == 1:
                            c0 = (ca + 1) * 128
                        else:
                            continue
                        nc.gpsimd.tensor_mul(
                            pt[:, c0:c0 + 128], pt[:, c0:c0 + 128],
                            mask_b,
                        )
                    if r == 0:
                        fill_one()
                        flush_av()
                    if prev_rnd is not None:
                        av8(prev_rnd[0], prev_rnd[1], 0)
                    if r > 0:
                        fill_one()
                    prev_rnd = (pt, gu)
                pend_av[0] = (lambda pt_=prev_rnd[0], gu_=prev_rnd[1]:
                              av8(pt_, gu_, 0))

            x_tiles = {}

            def emit_x_dma(c):
                x_h, x_l = [], []
                for pr in range(4):
                    xt_h = work.tile([128, 2, 512], E4, tag=f"xh{pr}", bufs=3,
                                     name=f"xh{c}_{pr}")
                    xt_l = work.tile([128, 2, 512], E4, tag=f"xl{pr}", bufs=3,
                                     name=f"xl{c}_{pr}")
                    # chunk 0 split across two queues so all 8 tiles beat the
                    # first qkv accumulation chain
                    heng = nc.sync if (c == 0 and pr >= 2) else nc.gpsimd
                    leng = nc.sync if (c == 0 and pr < 2) else nc.gpsimd
                    heng.dma_start(
                        xt_h, xh[128 * pr:128 * pr + 128,
                                 1024 * c:1024 * c + 1024].rearrange(
                            "p (two c) -> p two c", two=2))
                    leng.dma_start(
                        xt_l, xl[128 * pr:128 * pr + 128,
                                 1024 * c:1024 * c + 1024].rearrange(
                            "p (two c) -> p two c", two=2))
                    x_h.append(xt_h)
                    x_l.append(xt_l)
                x_tiles[c] = (x_h, x_l)

            chains_done = set()

            def emit_chain(c, m):
                if ("qk", c, m) in chains_done:
                    return
                chains_done.add(("qk", c, m))
                x_h, x_l = x_tiles[c]
                ps = psum.tile([128, 512], F32, tag="mm", bufs=2,
                               name=f"ps{c}_{m}")
                kk = 0
                for pr in range(4):
                    for (wt, xt) in ((wqk_h[m], x_h[pr]),
                                     (wqk_l[m], x_h[pr]),
                                     (wqk_h[m], x_l[pr])):
                        nc.tensor.matmul(
                            ps, wt[:, pr], xt,
                            start=(kk == 0), stop=(kk == 11),
                            perf_mode=DR,
                        )
                        kk += 1
                dst = q_sb[m] if m < 4 else k_t[m - 4]
                nc.vector.tensor_scalar_add(
                    dst[:, c * 512:(c + 1) * 512], ps,
                    bias_all[:, m:m + 1]
                )

            def emit_vchain(c, t4):
                if ("v", c, t4) in chains_done:
                    return
                chains_done.add(("v", c, t4))
                x_h, x_l = x_tiles[c]
                vtag, vbufs = (("acc", 1) if c == 0 and t4 == 2
                               else ("mm", 2))
                ps = psum.tile([128, 512], F32, tag=vtag,
                               bufs=vbufs, name=f"psv{c}_{t4}")
                kk = 0
                for pr in range(4):
                    xs_h = x_h[pr][:, :, t4 * 128:(t4 + 1) * 128]
                    xs_l = x_l[pr][:, :, t4 * 128:(t4 + 1) * 128]
                    for (lt, rt) in ((xs_h, wv_h[pr]),
                                     (xs_h, wv_l[pr]),
                                     (xs_l, wv_h[pr])):
                        nc.tensor.matmul(
                            ps, lt, rt,
                            start=(kk == 0), stop=(kk == 11),
                            perf_mode=DR,
                        )
                        kk += 1
                idx = 4 * c + t4
                if idx < 2 * PT8J:
                    nc.vector.tensor_copy(
                        v_store[idx].rearrange(
                            "p (g c) -> p g c", c=65)[:, :, 0:64],
                        ps.rearrange("p (g c) -> p g c", c=64),
                    )
                ip, sl = idx // 2, idx % 2
                hi_sl = vp_hi[ip][:, sl].rearrange(
                    "p (g c) -> p g c", c=65)[:, :, 0:64]
                lo_sl = vp_lo[ip][:, sl].rearrange(
                    "p (g c) -> p g c", c=65)[:, :, 0:64]
                ps_v = ps.rearrange("p (g c) -> p g c", c=64)
                # hi = e4(ps/64) = e4(16 v); lo = e4(ps/64 - hi)
                nc.vector.tensor_scalar_mul(hi_sl, ps_v, 1.0 / 64.0)
                nc.vector.scalar_tensor_tensor(
                    lo_sl, ps_v, 1.0 / 64.0, hi_sl,
                    mybir.AluOpType.mult, mybir.AluOpType.subtract,
                )

            # ---- diagonal-major unit schedule ----
            # A unit (c, p) = qkv chains for head-pair p of chunk c + that
            # pair's attention blocks.  The exp-rich pair-0 units of chunks
            # 2/3 are pulled EARLY (into the PE-bound phases of chunks 1/2)
            # so the ACT exp stream never runs dry during the causal ramp.
            UNITS = [(c, p) for c in range(4) for p in range(4)]
            # x DMAs issue well before each chunk's first unit
            X_PREFETCH = {(0, 0): [0], (0, 2): [1], (1, 1): [2], (2, 1): [3]}
            PSY_BUDGET = {}
            o_sbs = {}
            for ui, (c, p) in enumerate(UNITS):
                for cpre in X_PREFETCH.get((c, p), []):
                    emit_x_dma(cpre)
                    if cpre == 0:
                        # late-needed loads behind the first x chunk;
                        # bias/mask first (needed by the first bias-add)
                        nc.gpsimd.dma_start(bias_all, bqk[:, :])
                        nc.gpsimd.dma_start(mask_b, mask01[:, :])
                        for pr in range(4):
                            veng = nc.gpsimd if pr < 2 else nc.sync
                            veng.dma_start(
                                wv_h[pr],
                                wvh[128 * pr:128 * pr + 128, :].rearrange(
                                    "p (two c) -> p two c", two=2))
                            veng.dma_start(
                                wv_l[pr],
                                wvl[128 * pr:128 * pr + 128, :].rearrange(
                                    "p (two c) -> p two c", two=2))
                        for m in (2, 6):
                            nc.sync.dma_start(
                                wqk_h[m],
                                wqkh[128 * m:128 * m + 128, :].rearrange(
                                    "p (pr two c) -> p pr two c", two=2,
                                    c=128))
                            nc.sync.dma_start(
                                wqk_l[m],
                                wqkl[128 * m:128 * m + 128, :].rearrange(
                                    "p (pr two c) -> p pr two c", two=2,
                                    c=128))
                        for m in (3, 7):
                            nc.gpsimd.dma_start(
                                wqk_h[m],
                                wqkh[128 * m:128 * m + 128, :].rearrange(
                                    "p (pr two c) -> p pr two c", two=2,
                                    c=128))
                            nc.gpsimd.dma_start(
                                wqk_l[m],
                                wqkl[128 * m:128 * m + 128, :].rearrange(
                                    "p (pr two c) -> p pr two c", two=2,
                                    c=128))
                        for g in range(2):
                            nc.sync.dma_start(
                                wp_h[g],
                                wph[128 * g:128 * g + 128, :].rearrange(
                                    "p (two c) -> p two c", two=2))
                            nc.sync.dma_start(
                                wp_l[g],
                                wpl[128 * g:128 * g + 128, :].rearrange(
                                    "p (two c) -> p two c", two=2))
                if p == 0:
                    o_sbs[c] = {
                        j: [
                            work.tile([128, HGF], BF16, tag=f"os{s}", bufs=4,
                                      name=f"o{j}_{s}")
                            for s in range(2)
                        ]
                        for j in (2 * c, 2 * c + 1)
                    }
                o_sb = o_sbs[c]
                if (c, p) in PSY_BUDGET:
                    fill_state.update(budget=PSY_BUDGET[(c, p)], tick=0,
                                      rate=(2 if c < 3 else 1))
                if c == 3 and p == 2:
                    # j=6 heads 0-3 (cols 0:256) complete once (6,3)'s
                    # pending AV flushes: transpose/split that half early
                    post_flush[0] = lambda osb_=o_sb: (
                        emit_oT_half(osb_, 6, 0, 0, 2),
                        emit_oT_half(osb_, 6, 1, 0, 2))
                if c == 3 and p == 3:
                    # heads 4,5 (block 2) complete once (6,5) flushes
                    post_flush[0] = lambda osb_=o_sb: (
                        emit_oT_half(osb_, 6, 0, 2, 3),
                        emit_oT_half(osb_, 6, 1, 2, 3))
                for m in (p, 4 + p):  # q then k, transposed layout
                    emit_chain(c, m)
                if p == 0:
                    # V tiles 0,1 now; 2,3 ride between the j=2c and j=2c+1
                    # blocks (chunk 3 runs j=7 first and needs all four)
                    emit_vchain(c, 0)
                    emit_vchain(c, 1)
                    if c == 3:
                        emit_vchain(c, 2)
                        emit_vchain(c, 3)
                # chunk 3 runs j=7 before j=6 so j=7's o tiles finish
                # (and start their projections) while j=6 is still in
                # flight, shortening the closing chain
                if c == 3:
                    jh_list = [(7, 2 * p), (7, 2 * p + 1),
                               (6, 2 * p), (6, 2 * p + 1)]
                else:
                    jh_list = [(2 * c, 2 * p), (2 * c, 2 * p + 1),
                               (2 * c + 1, 2 * p), (2 * c + 1, 2 * p + 1)]
                for bi_jh, (j, h) in enumerate(jh_list):
                    if p == 0 and bi_jh == 2 and c < 3:
                        emit_vchain(c, 2)
                        emit_vchain(c, 3)
                    pair, off = h // 2, 64 * (h % 2)
                        if j >= PT8J:
                            emit_attn_block8(j, h, pair, off, o_sb)
                            if n == 3 and p == 3 and j == 7 and h == 7:
                                flush_av()
                                emit_oT_split(7, 0)
                                emit_oT_split(7, 1)
                            continue
                        acc2 = psum.tile([128, 512], F32, tag="acc", bufs=1,
                                         name=f"acc{j}_{h}")
                        acc = [acc2[:, 256 * s:256 * s + 65] for s in range(2)]
                        blocks = [(s, i) for s in range(2)
                                  for i in range(2 * j + s + 1)]
                        # software pipeline: each group's AV matmuls are
                        # emitted after the NEXT group's S matmuls (the AV
                        # waits on this group's exp; the next S does not), so
                        # the in-order PE always has exp-independent work
                        # while ACT exponentiates.  The last group's AV is
                        # carried into the next block via pend_av.
                        prev_grp = None
                        for g in range((len(blocks) + 7) // 8):
                            grp = blocks[8 * g:8 * g + 8]
                            sg = psum.tile([128, 1024 * RG], F32, tag="big",
                                           bufs=(2 if RG == 1 else 1),
                                           name=f"sg{j}_{h}_{g}")
                            for bi, (s, i) in enumerate(grp):
                                nc.tensor.matmul(
                                    sg[:, bi * 128:(bi + 1) * 128],
                                    k_t[pair][off:off + 64, i * 128:(i + 1) * 128],
                                    q_sb[pair][off:off + 64,
                                               j * QT + s * 128:
                                               j * QT + s * 128 + 128],
                                    start=True,
                                    stop=True,
                                )
                            pt = work.tile([128, 1024 * RG], BF16, tag="pt",
                                           bufs=2, name=f"pt{j}_{h}_{g}")
                            nc.scalar.activation(
                                pt[:, :len(grp) * 128], sg[:, :len(grp) * 128],
                                Exp, scale=EXP_SCALE
                            )
                            for bi, (s, i) in enumerate(grp):
                                if i == 2 * j + s:  # diagonal triangle
                                    nc.gpsimd.tensor_mul(
                                        pt[:, bi * 128:(bi + 1) * 128],
                                        pt[:, bi * 128:(bi + 1) * 128],
                                        mask_b,
                                    )
                            if g == 0:
                                flush_av()  # previous block's last AV + o
                            if prev_grp is not None:
                                emit_av(prev_grp[0], prev_grp[1], acc, j, h)
                            prev_grp = (pt, grp)
                        def _old_flush(pt_=prev_grp[0], grp_=prev_grp[1],
                                       acc_=acc, j_=j, h_=h, osb_=o_sb):
                            emit_av(pt_, grp_, acc_, j_, h_)
                            for s2 in range(2):
                                dinv = work.tile([128, 1], F32, tag="dinv",
                                                 bufs=4,
                                                 name=f"di{j_}_{h_}_{s2}")
                                nc.vector.reciprocal(dinv, acc_[s2][:, 64:65])
                                nc.vector.tensor_scalar_mul(
                                    osb_[j_][s2][:, 64 * h_:64 * h_ + 64],
                                    acc_[s2][:, 0:64],
                                    dinv,
                                )
                        pend_av[0] = _old_flush
                # transpose + hi/lo split for this chunk's finished o tiles
                flush_av()
                if n < 3:
                    for j in (2 * n, 2 * n + 1):
                        for s in range(2):
                            emit_oT_split(j, s)
                else:
                    for s in range(2):
                        emit_oT_half(6, s, 1)
                        _, oT_hi, oT_lo = oT_parts[(6, s)]
                        for nn in range(2):
                            ready_proj.append((6, s, oT_hi, oT_lo, nn))

            # ---- tail: remaining proj groups (j=6) on four distinct PSUM
            # banks so the chains run back-to-back and drains parallelize ----
            n_tail = len(ready_proj)
            tail_tags = [("py", 1), ("mm", 2), ("mm", 2), ("acc", 1)]
            for i, item in enumerate(ready_proj):
                tag, bufs = tail_tags[i % 4]
                drain = ("final" if i >= n_tail - 2 else
                         ("act" if i % 2 == 0 else "dve"))
                emit_psy(*item, tag=tag, bufs=bufs, drain=drain)

    nc.finalize()
    return nc


_NC = None


def _get_nc():
    global _NC
    if _NC is None:
        _NC = build_kernel()
    return _NC


def _hi_lo(a):
    """Split f32 array into e4m3 hi + lo (returned as ml_dtypes arrays)."""
    import ml_dtypes

    e4 = ml_dtypes.float8_e4m3fn
    hi = a.astype(e4)
    lo = (a - hi.astype(np.float32)).astype(e4)
    return hi, lo


def kernel(x, Wqkv, bqkv, Wproj, bproj, _trace=False):
    x = np.asarray(x, dtype=np.float32)
    Wqkv = np.asarray(Wqkv, dtype=np.float32)
    bqkv = np.asarray(bqkv, dtype=np.float32)
    Wproj = np.asarray(Wproj, dtype=np.float32)
    bproj = np.asarray(bproj, dtype=np.float32)

    import ml_dtypes

    bf16 = ml_dtypes.bfloat16
    # [key, query] diagonal triangle: allow key <= query
    mask = np.triu(np.ones((128, 128), dtype=np.float32)).astype(bf16)
    in_maps = []
    for hg in range(2):
        sl = slice(hg * HGF, (hg + 1) * HGF)
        rows = np.concatenate(
            [Wqkv[sl], Wqkv[1024 + hg * HGF:1024 + (hg + 1) * HGF],
             Wqkv[2048 + hg * HGF:2048 + (hg + 1) * HGF]]
        )
        wqkvT = np.ascontiguousarray(rows.T) * SB          # [C, 1536], 64x
        w_hi, w_lo = _hi_lo(wqkvT)
        # q/k part (cols 0:1024): m-tile pack [1024, 1024]
        #   row = 128*m + p holds W'[256*pr + 128*two + p, 128*m + c]
        def pack_qk(wa):
            blk = wa.astype(np.float32)[:, 0:1024]
            out = np.empty((1024, 1024), dtype=np.float32)
            for m in range(8):
                b4 = blk[:, 128 * m:128 * m + 128].reshape(4, 2, 128, 128)
                out[128 * m:128 * m + 128] = (
                    b4.transpose(2, 0, 1, 3).reshape(128, 1024)
                )
            return out

        # v part (cols 1024:1536): moving pack [512, 1024]
        #   row = 128*pr + p holds W'[256*pr + 128*two + p, 1024 + c]
        def pack_v(wa):
            blk = wa.astype(np.float32)[:, 1024:1536].reshape(4, 2, 128, 512)
            return blk.transpose(0, 2, 1, 3).reshape(512, 1024)

        e4 = ml_dtypes.float8_e4m3fn
        wqkh_np = np.ascontiguousarray(pack_qk(w_hi)).astype(e4)
        wqkl_np = np.ascontiguousarray(pack_qk(w_lo)).astype(e4)
        wvh_np = np.ascontiguousarray(pack_v(w_hi)).astype(e4)
        wvl_np = np.ascontiguousarray(pack_v(w_lo)).astype(e4)
        bq = np.ascontiguousarray(
            (np.concatenate(
                [bqkv[sl], bqkv[1024 + hg * HGF:1024 + (hg + 1) * HGF]]
            ) * (SA * SB)).reshape(8, 128).T
        ).astype(np.float32)
        # proj weights [512, 1024] o-feat rows, 64x; pack [256, 2048]:
        #   row = 128*g + p holds Wp'[256*g + 128*two + p, c]
        wprojT = np.ascontiguousarray(Wproj[:, sl].T) * SB
        wp_hi, wp_lo = _hi_lo(wprojT)

        def pack_wp(wa):
            blk = wa.astype(np.float32).reshape(2, 2, 128, 1024)
            return blk.transpose(0, 2, 1, 3).reshape(256, 2048)

        wph_np = np.ascontiguousarray(pack_wp(wp_hi)).astype(e4)
        wpl_np = np.ascontiguousarray(pack_wp(wp_lo)).astype(e4)
        for b in range(B):
            # x pack: [512, 4096]: row = 128*pr + p,
            # col = 1024*n + 512*two + t holds x'[256*pr + 128*two + p, t]
            xT = np.ascontiguousarray(x[b].T) * SA         # [C, T], 16x
            x_hi, x_lo = _hi_lo(xT)

            def pack_x(xa):
                blk = xa.astype(np.float32).reshape(4, 2, 128, 4, 512)
                return blk.transpose(0, 2, 3, 1, 4).reshape(512, 4096)

            in_maps.append(
                {
                    "xh": np.ascontiguousarray(pack_x(x_hi)).astype(e4),
                    "xl": np.ascontiguousarray(pack_x(x_lo)).astype(e4),
                    "wqkh": wqkh_np,
                    "wqkl": wqkl_np,
                    "wvh": wvh_np,
                    "wvl": wvl_np,
                    "bqk": bq,
                    "wph": wph_np,
                    "wpl": wpl_np,
                    "mask01": mask,
                }
            )
    # core order: idx = hg * 4 + b  (in_maps built hg-major already)
    in_maps = in_maps[:4] + in_maps[4:]
    res = run_bass_kernel_spmd(_get_nc(), in_maps, core_ids=list(range(8)),
                               trace=_trace)
    # V-bias folds into a constant output row: softmax rows sum to 1, so
    # y += (Wproj @ bv) for the full bv (both head groups combined)
    bias_row = bproj + Wproj @ bqkv[2 * C:3 * C]
    out = np.empty((B, T, C), dtype=np.float32)
    for b in range(B):
        out[b] = (res.results[b]["y"] + res.results[4 + b]["y"]) / OUT_DIV \
            + bias_row
    if _trace:
        return out, res
    return out


# revision 62
# speedup vs baseline: 1.0230x; 1.0230x over previous
"""Causal self-attention (B=4, T=2048, C=1024, NH=16) on 8 trn2 NeuronCores.

Sharding: core = (head_group hg in {0,1}) x (batch b in {0..3}).
Each core computes qkv projection + attention + partial output projection for
its 8 heads of its batch; host sums the two head-group partials per batch and
adds the output bias.

v3 = v2 + fp8 DoubleRow "3-slot split" for the qkv and output projections.
  - A matmul instruction costs out_free_size x cycles_per_row; fp8e4/e5 with
    perf_mode=DoubleRow runs at 0.5 cycles/row and contracts TWO 128-row
    k-tiles per instruction (lhsT [128,2,M], rhs [128,2,N]).  Writing
    X = Xh + Xl and W = Wh + Wl (each an e4m3 pair: hi = e4(x), lo =
    e4(x - hi), together ~9 mantissa bits > bf16's 8), the product
    X@W ~= Xh@Wh + Xh@Wl + Xl@Wh needs 3 slot-products per k-tile = 1.5
    DoubleRow instructions per k-tile pair = 0.75x the bf16 PE cost.
    Measured end-to-end rel-err 4.3e-3 vs bf16's 5.6e-3 (the e4m3 pair is
    slightly MORE precise than bf16).
  - Scale scheme keeps every fp8 operand in e4m3 normal range with all
    compensation factors exact powers of two: x' = 16x, W' = 64Wqkv =>
    PSUM q/k/v are 1024x; bias ships 1024x; scores are 2^20 x so the exp
    scale is 0.125*2^-20; the AV ones-column is 64.0 so o = acc*dinv comes
    out 16x; Wproj' = 64Wproj => y partials are 1024x, divided on the host.
  - qkv/V weights and x ship as host-packed hi/lo e4m3 pairs in DR-friendly
    row order (slot pairs contiguous), so SBUF tiles load with plain DMAs:
    same total bytes as the bf16 v2 (hi+lo = 2 bytes/elem).
  - o (= 16x true o, bf16) transposes to oT exactly as v2 (one
    dma_start_transpose per 128-query chunk), then splits on DVE into
    oT_hi = e4(oT), oT_lo = e4(oT - oT_hi) for the deferred DR projection.
  - scores and AV stay bf16: the score contraction is only 64 (no k-tile
    pair to fuse) and AV's pt residual would need a second exp pass.

Attention core (unchanged from v2):
  - q/k computed transposed (head_size on partitions); V in natural
    [token, feat] layout; V bias folded into the host-side output bias.
  - S^T = K @ Q^T per (head, 128-key block, 128-query chunk); 8 causal
    blocks packed in one [128, 1024] PSUM tile so one ScalarE Exp covers
    them -- and for j >= PT8J BOTH heads of a pair stream through shared
    groups (pair-merged: 144 exp instructions instead of 160; acc tiles
    allocate lazily per head, each head's dinv/o-scale emits the moment
    its last AV lands so the single acc bank recycles mid-group).
    Causal mask = one 0/1 multiply per diagonal block on GpSimd.
  - AV runs query-on-partitions: acc[q, 0:65] += pt_slice^T @ [V | 64];
    the softmax denominator arrives as a per-partition scalar -> DVE
    reciprocal + tensor_scalar_mul.
  - output projections deferred to the end of the program as PE fill for
    the ACT-bound late tiles.
  - a few throwaway warmup matmuls burn the PE p-state ramp.
Cost-model span: 180580 ns/core (v2 bf16: 203588; original stub: 309000).
Engine busy: PE 160.3us, ACT 151.9us, DVE 105.2us, Pool 45.0us, SP 29.2us.
"""

import sys

sys.path.insert(0, "/opt/trn_rl_repo")

import numpy as np

import concourse.bacc as bacc
import concourse.bass as bass
import concourse.mybir as mybir
from concourse.bass_utils import run_bass_kernel_spmd
from concourse.tile import TileContext

B, T, C, NH = 4, 2048, 1024, 16
HS = C // NH          # 64
HGF = 512             # features per head group (8 heads x 64)
QT = 256              # query tile (S stage)
NKT = T // 128        # 16 key tiles
F32 = mybir.dt.float32
BF16 = mybir.dt.bfloat16
E4 = mybir.dt.float8e4
DR = mybir.MatmulPerfMode.DoubleRow
Exp = mybir.ActivationFunctionType.Exp

SA = 16.0             # x scale
SB = 64.0             # weight scale
EXP_SCALE = 0.125 / float(SA * SA * SB * SB)   # 0.125 * 2^-20
ONES_VAL = 64.0       # AV denominator column value => o = 16x true o
OUT_DIV = SA * SB     # host divides y partials by 1024
# query chunks j >= PT8J run attention weights in e4m3: exp emits pt8 =
# exp(s - PT8_SHIFT) (max scaled score is 9.04 -> pt8 <= e^5.04 = 155 < 448;
# rows have >= 513 keys so a row of all-zero pt8 is impossible), V ships as
# a 16x e4m3 hi/lo pair, and AV runs DoubleRow over key-tile pairs at half
# the bf16 PE cost.  The pt8 denominator column (1.0) normalizes with the
# same quantized weights, cancelling most of the quantization error.
PT8J = 2
PT8_SHIFT = -4.0
RG = 1                # S-groups fused per exp instruction (1 = double-buffered)


def build_kernel():
    nc = bacc.Bacc(None, target_bir_lowering=False)
    # packed x: row = 128*pr + p, col = 1024*n + 512*two + t
    xh = nc.dram_tensor("xh", (512, 4096), E4, kind="ExternalInput")
    xl = nc.dram_tensor("xl", (512, 4096), E4, kind="ExternalInput")
    # packed q/k weights: row = 128*m + p (m 0..3 q, 4..7 k),
    # col = 256*pr + 128*two + c
    wqkh = nc.dram_tensor("wqkh", (1024, 1024), E4, kind="ExternalInput")
    wqkl = nc.dram_tensor("wqkl", (1024, 1024), E4, kind="ExternalInput")
    # packed v weights (moving layout): row = 128*pr + p, col = 512*two + c
    wvh = nc.dram_tensor("wvh", (512, 1024), E4, kind="ExternalInput")
    wvl = nc.dram_tensor("wvl", (512, 1024), E4, kind="ExternalInput")
    bqk = nc.dram_tensor("bqk", (128, 8), F32, kind="ExternalInput")
    # packed proj weights: row = 128*g + p, col = 1024*two + c
    wph = nc.dram_tensor("wph", (256, 2048), E4, kind="ExternalInput")
    wpl = nc.dram_tensor("wpl", (256, 2048), E4, kind="ExternalInput")
    mask01 = nc.dram_tensor("mask01", (128, 128), BF16, kind="ExternalInput")
    y = nc.dram_tensor("y", (T, C), F32, kind="ExternalOutput")

    with TileContext(nc) as tc:
        with (
            tc.tile_pool(name="outer", bufs=1) as outer,
            tc.tile_pool(name="work", bufs=1) as work,
            tc.tile_pool(name="psum", bufs=1, space="PSUM") as psum,
        ):
            # ---- PE p-state warmup ----
            warm = outer.tile([128, 512], BF16, name="warm")
            nc.vector.memset(warm, 0.0)
            for wi in range(5):
                wtag, wbufs = ("py", 1) if wi % 2 == 0 else ("mm", 2)
                wps = psum.tile([128, 512], F32, tag=wtag, bufs=wbufs,
                                name=f"warm{wi}")
                nc.tensor.matmul(wps, warm[:, 0:128], warm,
                                 start=True, stop=True)

            bias_all = outer.tile([128, 8], F32, name="bias_all")
            mask_b = outer.tile([128, 128], BF16, name="mask_b")
            # q/k weight m-tiles [p, pr, two, c]; q part (m<4) loads first
            # on the idle scalar queue so the first qkv chain isn't blocked
            wqk_h = [outer.tile([128, 4, 2, 128], E4, name=f"wqh{m}")
                     for m in range(8)]
            wqk_l = [outer.tile([128, 4, 2, 128], E4, name=f"wql{m}")
                     for m in range(8)]
            # pair-priority order: the first S block needs q m0 AND k m4.
            # Only pairs 0/1 ride the scalar queue: its DMA issues occupy
            # the ACT sequencer ~500ns each and would otherwise queue-block
            # the first exp until ~8us.  Pairs 2/3 load later via sync /
            # gpsimd (emitted in the chunk-0 extras below).
            for m in (0, 4, 1, 5):
                nc.scalar.dma_start(
                    wqk_h[m], wqkh[128 * m:128 * m + 128, :].rearrange(
                        "p (pr two c) -> p pr two c", two=2, c=128))
                nc.scalar.dma_start(
                    wqk_l[m], wqkl[128 * m:128 * m + 128, :].rearrange(
                        "p (pr two c) -> p pr two c", two=2, c=128))
            wv_h = [outer.tile([128, 2, 512], E4, name=f"wvh{pr}")
                    for pr in range(4)]
            wv_l = [outer.tile([128, 2, 512], E4, name=f"wvl{pr}")
                    for pr in range(4)]
            wp_h = [outer.tile([128, 2, 1024], E4, name=f"wph{g}")
                    for g in range(2)]
            wp_l = [outer.tile([128, 2, 1024], E4, name=f"wpl{g}")
                    for g in range(2)]

            k_t = [outer.tile([128, T], BF16, name=f"k{i}") for i in range(4)]
            q_sb = [outer.tile([128, T], BF16, name=f"q{i}") for i in range(4)]
            # v_store[i] (bf16, j < PT8J only): [key-tile 128, 8*65]; per head
            # h cols 65h:65h+64 are V features (1024x), col 65h+64 is
            # ONES_VAL (softmax denominator)
            v_store = [outer.tile([128, 8 * 65], BF16, name=f"v{i}")
                       for i in range(2 * PT8J)]
            for i in range(2 * PT8J):
                nc.vector.memset(
                    v_store[i].rearrange("p (g c) -> p g c", c=65)[:, :, 64:65],
                    ONES_VAL,
                )
            # vp_hi/vp_lo[ip] (e4m3, j >= PT8J): key-tile PAIR (2ip, 2ip+1)
            # as DoubleRow slots; per head h cols 65h:65h+64 hold 16x V
            # hi/lo, col 65h+64 is 1.0 in hi / 0.0 in lo (denominator)
            vp_hi = [outer.tile([128, 2, 8 * 65], E4, name=f"vph{ip}")
                     for ip in range(NKT // 2)]
            vp_lo = [outer.tile([128, 2, 8 * 65], E4, name=f"vpl{ip}")
                     for ip in range(NKT // 2)]
            for ip in range(NKT // 2):
                nc.vector.memset(
                    vp_hi[ip].rearrange("p t (g c) -> p t g c", c=65)[:, :, :, 64:65],
                    1.0,
                )
                nc.vector.memset(
                    vp_lo[ip].rearrange("p t (g c) -> p t g c", c=65)[:, :, :, 64:65],
                    0.0,
                )
            nb_sh = outer.tile([128, 1], F32, name="nb_sh")
            nc.vector.memset(nb_sh, PT8_SHIFT)

            # ---- deferred/interleaved output projection machinery ----
            # ready_proj holds (j, s, oT_hi, oT_lo, nn) psy groups whose oT
            # splits are already emitted; emit_psy pops them into the PE
            # stream.  Interleaved groups drain on DVE (ACT is exp-saturated
            # in the late chunks); tail groups split drains as in v2.
            ready_proj = []
            psy_count = [0]

            def emit_psy(j, s, oT_hi, oT_lo, nn, tag, bufs, drain):
                psy = psum.tile([128, 512], F32, tag=tag, bufs=bufs,
                                name=f"py{j}_{s}_{nn}")
                kk = 0
                for g in range(2):
                    for (lt, rt) in ((oT_hi, wp_h[g]), (oT_lo, wp_h[g]),
                                     (oT_hi, wp_l[g])):
                        nc.tensor.matmul(
                            psy,
                            lt[:, 2 * g:2 * g + 2, :],
                            rt[:, :, nn * 512:(nn + 1) * 512],
                            start=(kk == 0), stop=(kk == 5),
                            perf_mode=DR,
                        )
                        kk += 1
                ysb = work.tile([128, 512], F32, tag="ysb", bufs=6,
                                name=f"ys{j}_{s}_{nn}")
                rows = y[j * QT + s * 128:j * QT + (s + 1) * 128, :]
                if drain == "final":
                    # final drain: halves in parallel on two engines and
                    # four DMA queues to shorten the closing chain
                    nc.scalar.activation(
                        ysb[:, 0:256], psy[:, 0:256],
                        mybir.ActivationFunctionType.Copy,
                    )
                    nc.vector.tensor_copy(ysb[:, 256:512], psy[:, 256:512])
                    q0, q1 = ((nc.sync, nc.gpsimd) if psy_count[0] % 2 == 0
                              else (nc.scalar, nc.sync))
                    q0.dma_start(
                        rows[:, nn * 512:nn * 512 + 256], ysb[:, 0:256]
                    )
                    q1.dma_start(
                        rows[:, nn * 512 + 256:nn * 512 + 512],
                        ysb[:, 256:512],
                    )
                else:
                    if drain == "act":
                        nc.scalar.activation(
                            ysb, psy, mybir.ActivationFunctionType.Copy
                        )
                    else:
                        nc.vector.tensor_copy(ysb, psy)
                    yeng = nc.sync if psy_count[0] % 2 == 0 else nc.gpsimd
                    yeng.dma_start(rows[:, nn * 512:(nn + 1) * 512], ysb)
                psy_count[0] += 1

            # pend_av carries the last S-group of a block whose AV matmuls
            # (exp-dependent) haven't been emitted yet; flush_av emits them
            # plus the block's dinv / o-scale ops.
            pend_av = [None]
            # filler machinery: one deferred-proj psy group per attention
            # round gives the in-order PE exp-independent work while ACT
            # drains the single-buffered S tile
            fill_state = {"budget": 0, "rate": 1, "tick": 0}
            fillers = []

            def fill_one():
                if fillers:
                    fillers.pop(0)()
                    return
                fill_state["tick"] += 1
                if (fill_state["budget"] > 0 and ready_proj
                        and fill_state["tick"] % fill_state["rate"] == 0):
                    fill_state["budget"] -= 1
                    emit_psy(*ready_proj.pop(0), tag="py", bufs=1,
                             drain="dve")

            def emit_av(pt_, grp_, acc_, j_, h_):
                for bi, (s, i) in enumerate(grp_):
                    nc.tensor.matmul(
                        acc_[s],
                        pt_[:, bi * 128:(bi + 1) * 128],
                        v_store[i][:, 65 * h_:65 * h_ + 65],
                        start=(i == 0),
                        stop=(i == 2 * j_ + s),
                    )

            post_flush = [None]

            def flush_av():
                if pend_av[0] is None:
                    return
                fn = pend_av[0]
                pend_av[0] = None
                fn()
                if post_flush[0] is not None:
                    fn2 = post_flush[0]
                    post_flush[0] = None
                    fn2()

            oT_parts = {}

            def emit_oT_half(osb, j, s, b0, b1):
                # transpose + split feature blocks [b0, b1) (128 feats each,
                # 2 heads per block) of o tile (j, s); pieces of the last
                # chunk's tiles go out as soon as their heads complete,
                # shortening the closing chain
                if (j, s) not in oT_parts:
                    oT_parts[(j, s)] = (
                        work.tile([128, 4, 128], BF16, tag="ot", bufs=4,
                                  name=f"ot{j}_{s}"),
                        work.tile([128, 4, 128], E4, tag="oth", bufs=16,
                                  name=f"oth{j}_{s}"),
                        work.tile([128, 4, 128], E4, tag="otl", bufs=16,
                                  name=f"otl{j}_{s}"),
                    )
                oT, oT_hi, oT_lo = oT_parts[(j, s)]
                nc.sync.dma_start_transpose(
                    oT[:, b0:b1, :], osb[j][s][:, 128 * b0:128 * b1])
                nc.vector.tensor_copy(oT_hi[:, b0:b1, :], oT[:, b0:b1, :])
                nc.vector.tensor_tensor(
                    oT_lo[:, b0:b1, :], oT[:, b0:b1, :], oT_hi[:, b0:b1, :],
                    mybir.AluOpType.subtract)

            def emit_oT_split(osb, j, s):
                oT = work.tile([128, 4, 128], BF16, tag="ot", bufs=4,
                               name=f"ot{j}_{s}")
                nc.sync.dma_start_transpose(oT, osb[j][s])
                oT_hi = work.tile([128, 4, 128], E4, tag="oth",
                                  bufs=16, name=f"oth{j}_{s}")
                oT_lo = work.tile([128, 4, 128], E4, tag="otl",
                                  bufs=16, name=f"otl{j}_{s}")
                nc.vector.tensor_copy(oT_hi, oT)
                nc.vector.tensor_tensor(
                    oT_lo, oT, oT_hi, mybir.AluOpType.subtract)
                for nn in range(2):
                    ready_proj.append((j, s, oT_hi, oT_lo, nn))

            def emit_attn_pair8(j, p, osb):
                """Pair-merged attention for j >= PT8J: BOTH heads of pair p
                stream through shared S-groups so one exp instruction covers
                up to 8 blocks regardless of the head boundary (160 -> 144
                exps).  acc tiles allocate lazily per head inside the AV
                stream; a head's dinv/o-scale emit the moment its last AV
                lands, releasing the single acc bank for the next head.
                """
                pair = p
                units = []   # (h, s, a, b); b None = s=0 diagonal single
                for h in (2 * p, 2 * p + 1):
                    for a in range(0, 2 * j, 2):
                        units.append((h, 0, a, a + 1))
                    units.append((h, 0, 2 * j, None))
                    for a in range(0, 2 * j + 2, 2):
                        units.append((h, 1, a, a + 1))
                groups, cur, cnt = [], [], 0
                for u in units:
                    w = 1 if u[3] is None else 2
                    if cnt + w > 8:
                        groups.append(cur)
                        cur, cnt = [], 0
                    cur.append((u, cnt))
                    cnt += w
                if cur:
                    groups.append(cur)
                accs = {}

                def av8(pt_, gu, base):
                    for (h, sx, a, b), ca in gu:
                        if h not in accs:
                            acc2 = psum.tile([128, 512], F32, tag="acc",
                                             bufs=1, name=f"acc{j}_{h}")
                            accs[h] = [acc2[:, 256 * s2:256 * s2 + 65]
                                       for s2 in range(2)]
                        acc_ = accs[h]
                        c0 = base + ca * 128
                        st = (a == 0)
                        if b is None:  # s=0 diagonal single: plain fp8 x2
                            for vt, last in ((vp_hi, False), (vp_lo, True)):
                                nc.tensor.matmul(
                                    acc_[sx],
                                    pt_[:, c0:c0 + 128],
                                    vt[a // 2][:, a % 2, 65 * h:65 * h + 65],
                                    start=False,
                                    stop=(sx == 0 and last),
                                )
                        else:
                            ptp = pt_[:, c0:c0 + 256].rearrange(
                                "p (two c) -> p two c", two=2)
                            for vt, last in ((vp_hi, False), (vp_lo, True)):
                                nc.tensor.matmul(
                                    acc_[sx], ptp,
                                    vt[a // 2][:, :, 65 * h:65 * h + 65],
                                    start=(st and vt is vp_hi),
                                    stop=(sx == 1 and b == 2 * j + 1
                                          and last),
                                    perf_mode=DR,
                                )
                        if sx == 1 and b == 2 * j + 1:
                            # head h complete: normalize + release its acc
                            for s2 in range(2):
                                dinv = work.tile([128, 1], F32, tag="dinv",
                                                 bufs=4,
                                                 name=f"di{j}_{h}_{s2}")
                                nc.vector.reciprocal(dinv, acc_[s2][:, 64:65])
                                nc.vector.tensor_scalar_mul(
                                    osb[j][s2][:, 64 * h:64 * h + 64],
                                    acc_[s2][:, 0:64],
                                    dinv,
                                )
                            del accs[h]

                prev_rnd = None
                for r, gu in enumerate(groups):
                    ntot = sum(1 if u[3] is None else 2 for u, _ in gu) * 128
                    sg = psum.tile([128, 1024], F32, tag="big", bufs=2,
                                   name=f"sg{j}_{p}_{r}")
                    for (h, sx, a, b), ca in gu:
                        off = 64 * (h % 2)
                        for bi, i in enumerate([a] if b is None else [a, b]):
                            c0 = (ca + bi) * 128
                            nc.tensor.matmul(
                                sg[:, c0:c0 + 128],
                                k_t[pair][off:off + 64,
                                          i * 128:(i + 1) * 128],
                                q_sb[pair][off:off + 64,
                                           j * QT + sx * 128:
                                           j * QT + sx * 128 + 128],
                                start=True,
                                stop=True,
                            )
                    pt = work.tile([128, 1024], E4, tag="p8", bufs=6,
                                   name=f"p8{j}_{p}_{r}")
                    nc.scalar.activation(
                        pt[:, :ntot], sg[:, :ntot],
                        Exp, scale=EXP_SCALE, bias=nb_sh,
                    )
                    for (h, sx, a, b), ca in gu:
                        if b is None and a == 2 * j:
                            c0 = ca * 128
                        elif b == 2 * j + 1 and sx == 1:
                            c0 = (ca + 1) * 128
                        else:
                            continue
                        nc.gpsimd.tensor_mul(
                            pt[:, c0:c0 + 128], pt[:, c0:c0 + 128],
                            mask_b,
                        )
                    if r == 0:
                        fill_one()
                        flush_av()
                    if prev_rnd is not None:
                        av8(prev_rnd[0], prev_rnd[1], 0)
                    if r > 0:
                        fill_one()
                    prev_rnd = (pt, gu)
                pend_av[0] = (lambda pt_=prev_rnd[0], gu_=prev_rnd[1]:
                              av8(pt_, gu_, 0))

            x_tiles = {}

            def emit_x_dma(c):
                x_h, x_l = [], []
                for pr in range(4):
                    xt_h = work.tile([128, 2, 512], E4, tag=f"xh{pr}", bufs=3,
                                     name=f"xh{c}_{pr}")
                    xt_l = work.tile([128, 2, 512], E4, tag=f"xl{pr}", bufs=3,
                                     name=f"xl{c}_{pr}")
                    # chunk 0 split across two queues so all 8 tiles beat the
                    # first qkv accumulation chain
                    heng = nc.sync if (c == 0 and pr >= 2) else nc.gpsimd
                    leng = nc.sync if (c == 0 and pr < 2) else nc.gpsimd
                    heng.dma_start(
                        xt_h, xh[128 * pr:128 * pr + 128,
                                 1024 * c:1024 * c + 1024].rearrange(
                            "p (two c) -> p two c", two=2))
                    leng.dma_start(
                        xt_l, xl[128 * pr:128 * pr + 128,
                                 1024 * c:1024 * c + 1024].rearrange(
                            "p (two c) -> p two c", two=2))
                    x_h.append(xt_h)
                    x_l.append(xt_l)
                x_tiles[c] = (x_h, x_l)

            chains_done = set()

            def emit_chain(c, m):
                if ("qk", c, m) in chains_done:
                    return
                chains_done.add(("qk", c, m))
                x_h, x_l = x_tiles[c]
                ps = psum.tile([128, 512], F32, tag="mm", bufs=2,
                               name=f"ps{c}_{m}")
                kk = 0
                for pr in range(4):
                    for (wt, xt) in ((wqk_h[m], x_h[pr]),
                                     (wqk_l[m], x_h[pr]),
                                     (wqk_h[m], x_l[pr])):
                        nc.tensor.matmul(
                            ps, wt[:, pr], xt,
                            start=(kk == 0), stop=(kk == 11),
                            perf_mode=DR,
                        )
                        kk += 1
                dst = q_sb[m] if m < 4 else k_t[m - 4]
                nc.vector.tensor_scalar_add(
                    dst[:, c * 512:(c + 1) * 512], ps,
                    bias_all[:, m:m + 1]
                )

            def emit_vchain(c, t4):
                if ("v", c, t4) in chains_done:
                    return
                chains_done.add(("v", c, t4))
                x_h, x_l = x_tiles[c]
                vtag, vbufs = (("acc", 1) if c == 0 and t4 == 2
                               else ("mm", 2))
                ps = psum.tile([128, 512], F32, tag=vtag,
                               bufs=vbufs, name=f"psv{c}_{t4}")
                kk = 0
                for pr in range(4):
                    xs_h = x_h[pr][:, :, t4 * 128:(t4 + 1) * 128]
                    xs_l = x_l[pr][:, :, t4 * 128:(t4 + 1) * 128]
                    for (lt, rt) in ((xs_h, wv_h[pr]),
                                     (xs_h, wv_l[pr]),
                                     (xs_l, wv_h[pr])):
                        nc.tensor.matmul(
                            ps, lt, rt,
                            start=(kk == 0), stop=(kk == 11),
                            perf_mode=DR,
                        )
                        kk += 1
                idx = 4 * c + t4
                if idx < 2 * PT8J:
                    nc.vector.tensor_copy(
                        v_store[idx].rearrange(
                            "p (g c) -> p g c", c=65)[:, :, 0:64],
                        ps.rearrange("p (g c) -> p g c", c=64),
                    )
                ip, sl = idx // 2, idx % 2
                hi_sl = vp_hi[ip][:, sl].rearrange(
                    "p (g c) -> p g c", c=65)[:, :, 0:64]
                lo_sl = vp_lo[ip][:, sl].rearrange(
                    "p (g c) -> p g c", c=65)[:, :, 0:64]
                ps_v = ps.rearrange("p (g c) -> p g c", c=64)
                # hi = e4(ps/64) = e4(16 v); lo = e4(ps/64 - hi)
                nc.vector.tensor_scalar_mul(hi_sl, ps_v, 1.0 / 64.0)
                nc.vector.scalar_tensor_tensor(
                    lo_sl, ps_v, 1.0 / 64.0, hi_sl,
                    mybir.AluOpType.mult, mybir.AluOpType.subtract,
                )

            # ---- diagonal-major unit schedule ----
            # A unit (c, p) = qkv chains for head-pair p of chunk c + that
            # pair's attention blocks.  The exp-rich pair-0 units of chunks
            # 2/3 are pulled EARLY (into the PE-bound phases of chunks 1/2)
            # so the ACT exp stream never runs dry during the causal ramp.
            UNITS = [(c, p) for c in range(4) for p in range(4)]
            # x DMAs issue well before each chunk's first unit
            X_PREFETCH = {(0, 0): [0], (0, 2): [1], (1, 1): [2], (2, 1): [3]}
            PSY_BUDGET = {}
            o_sbs = {}
            for ui, (c, p) in enumerate(UNITS):
                for cpre in X_PREFETCH.get((c, p), []):
                    emit_x_dma(cpre)
                    if cpre == 0:
                        # late-needed loads behind the first x chunk;
                        # bias/mask first (needed by the first bias-add)
                        nc.gpsimd.dma_start(bias_all, bqk[:, :])
                        nc.gpsimd.dma_start(mask_b, mask01[:, :])
                        for pr in range(4):
                            veng = nc.gpsimd if pr < 2 else nc.sync
                            veng.dma_start(
                                wv_h[pr],
                                wvh[128 * pr:128 * pr + 128, :].rearrange(
                                    "p (two c) -> p two c", two=2))
                            veng.dma_start(
                                wv_l[pr],
                                wvl[128 * pr:128 * pr + 128, :].rearrange(
                                    "p (two c) -> p two c", two=2))
                        for m in (2, 6):
                            nc.sync.dma_start(
                                wqk_h[m],
                                wqkh[128 * m:128 * m + 128, :].rearrange(
                                    "p (pr two c) -> p pr two c", two=2,
                                    c=128))
                            nc.sync.dma_start(
                                wqk_l[m],
                                wqkl[128 * m:128 * m + 128, :].rearrange(
                                    "p (pr two c) -> p pr two c", two=2,
                                    c=128))
                        for m in (3, 7):
                            nc.gpsimd.dma_start(
                                wqk_h[m],
                                wqkh[128 * m:128 * m + 128, :].rearrange(
                                    "p (pr two c) -> p pr two c", two=2,
                                    c=128))
                            nc.gpsimd.dma_start(
                                wqk_l[m],
                                wqkl[128 * m:128 * m + 128, :].rearrange(
                                    "p (pr two c) -> p pr two c", two=2,
                                    c=128))
                        for g in range(2):
                            nc.sync.dma_start(
                                wp_h[g],
                                wph[128 * g:128 * g + 128, :].rearrange(
                                    "p (two c) -> p two c", two=2))
                            nc.sync.dma_start(
                                wp_l[g],
                                wpl[128 * g:128 * g + 128, :].rearrange(
                                    "p (two c) -> p two c", two=2))
                if p == 0:
                    o_sbs[c] = {
                        j: [
                            work.tile([128, HGF], BF16, tag=f"os{s}", bufs=4,
                                      name=f"o{j}_{s}")
                            for s in range(2)
                        ]
                        for j in (2 * c, 2 * c + 1)
                    }
                o_sb = o_sbs[c]
                if (c, p) in PSY_BUDGET:
                    fill_state.update(budget=PSY_BUDGET[(c, p)], tick=0,
                                      rate=(2 if c < 3 else 1))
                if c == 3 and p == 2:
                    # j=6 heads 0-3 (cols 0:256) complete once (6,3)'s
                    # pending AV flushes: transpose/split that half early
                    post_flush[0] = lambda osb_=o_sb: (
                        emit_oT_half(osb_, 6, 0, 0, 2),
                        emit_oT_half(osb_, 6, 1, 0, 2))
                if c == 3 and p == 3:
                    # heads 4,5 (block 2) complete once (6,5) flushes
                    post_flush[0] = lambda osb_=o_sb: (
                        emit_oT_half(osb_, 6, 0, 2, 3),
                        emit_oT_half(osb_, 6, 1, 2, 3))
                for m in (p, 4 + p):  # q then k, transposed layout
                    emit_chain(c, m)
                if p == 0:
                    # V tiles 0,1 now; 2,3 ride between the j=2c and j=2c+1
                    # blocks (chunk 3 runs j=7 first and needs all four)
                    emit_vchain(c, 0)
                    emit_vchain(c, 1)
                    if c == 3:
                        emit_vchain(c, 2)
                        emit_vchain(c, 3)
                # chunk 3 runs j=7 before j=6 so j=7's o tiles finish
                # (and start their projections) while j=6 is still in
                # flight, shortening the closing chain
                if c == 3:
                    jh_list = [(7, 2 * p), (7, 2 * p + 1),
                               (6, 2 * p), (6, 2 * p + 1)]
                else:
                    jh_list = [(2 * c, 2 * p), (2 * c, 2 * p + 1),
                               (2 * c + 1, 2 * p), (2 * c + 1, 2 * p + 1)]
                for bi_jh, (j, h) in enumerate(jh_list):
                    if p == 0 and bi_jh == 2 and c < 3:
                        emit_vchain(c, 2)
                        emit_vchain(c, 3)
                    pair, off = h // 2, 64 * (h % 2)
                        if j >= PT8J:
                            emit_attn_block8(j, h, pair, off, o_sb)
                            if n == 3 and p == 3 and j == 7 and h == 7:
                                flush_av()
                                emit_oT_split(7, 0)
                                emit_oT_split(7, 1)
                            continue
                        acc2 = psum.tile([128, 512], F32, tag="acc", bufs=1,
                                         name=f"acc{j}_{h}")
                        acc = [acc2[:, 256 * s:256 * s + 65] for s in range(2)]
                        blocks = [(s, i) for s in range(2)
                                  for i in range(2 * j + s + 1)]
                        # software pipeline: each group's AV matmuls are
                        # emitted after the NEXT group's S matmuls (the AV
                        # waits on this group's exp; the next S does not), so
                        # the in-order PE always has exp-independent work
                        # while ACT exponentiates.  The last group's AV is
                        # carried into the next block via pend_av.
                        prev_grp = None
                        for g in range((len(blocks) + 7) // 8):
                            grp = blocks[8 * g:8 * g + 8]
                            sg = psum.tile([128, 1024 * RG], F32, tag="big",
                                           bufs=(2 if RG == 1 else 1),
                                           name=f"sg{j}_{h}_{g}")
                            for bi, (s, i) in enumerate(grp):
                                nc.tensor.matmul(
                                    sg[:, bi * 128:(bi + 1) * 128],
                                    k_t[pair][off:off + 64, i * 128:(i + 1) * 128],
                                    q_sb[pair][off:off + 64,
                                               j * QT + s * 128:
                                               j * QT + s * 128 + 128],
                                    start=True,
                                    stop=True,
                                )
                            pt = work.tile([128, 1024 * RG], BF16, tag="pt",
                                           bufs=2, name=f"pt{j}_{h}_{g}")
                            nc.scalar.activation(
                                pt[:, :len(grp) * 128], sg[:, :len(grp) * 128],
                                Exp, scale=EXP_SCALE
                            )
                            for bi, (s, i) in enumerate(grp):
                                if i == 2 * j + s:  # diagonal triangle
                                    nc.gpsimd.tensor_mul(
                                        pt[:, bi * 128:(bi + 1) * 128],
                                        pt[:, bi * 128:(bi + 1) * 128],
                                        mask_b,
                                    )
                            if g == 0:
                                flush_av()  # previous block's last AV + o
                            if prev_grp is not None:
                                emit_av(prev_grp[0], prev_grp[1], acc, j, h)
                            prev_grp = (pt, grp)
                        def _old_flush(pt_=prev_grp[0], grp_=prev_grp[1],
                                       acc_=acc, j_=j, h_=h, osb_=o_sb):
                            emit_av(pt_, grp_, acc_, j_, h_)
                            for s2 in range(2):
                                dinv = work.tile([128, 1], F32, tag="dinv",
                                                 bufs=4,
                                                 name=f"di{j_}_{h_}_{s2}")
                                nc.vector.reciprocal(dinv, acc_[s2][:, 64:65])
                                nc.vector.tensor_scalar_mul(
                                    osb_[j_][s2][:, 64 * h_:64 * h_ + 64],
                                    acc_[s2][:, 0:64],
                                    dinv,
                                )
                        pend_av[0] = _old_flush
                # transpose + hi/lo split for this chunk's finished o tiles
                flush_av()
                if n < 3:
                    for j in (2 * n, 2 * n + 1):
                        for s in range(2):
                            emit_oT_split(j, s)
                else:
                    for s in range(2):
                        emit_oT_half(6, s, 1)
                        _, oT_hi, oT_lo = oT_parts[(6, s)]
                        for nn in range(2):
                            ready_proj.append((6, s, oT_hi, oT_lo, nn))

            # ---- tail: remaining proj groups (j=6) on four distinct PSUM
            # banks so the chains run back-to-back and drains parallelize ----
            n_tail = len(ready_proj)
            tail_tags = [("py", 1), ("mm", 2), ("mm", 2), ("acc", 1)]
            for i, item in enumerate(ready_proj):
                tag, bufs = tail_tags[i % 4]
                drain = ("final" if i >= n_tail - 2 else
                         ("act" if i % 2 == 0 else "dve"))
                emit_psy(*item, tag=tag, bufs=bufs, drain=drain)

    nc.finalize()
    return nc


_NC = None


def _get_nc():
    global _NC
    if _NC is None:
        _NC = build_kernel()
    return _NC


def _hi_lo(a):
    """Split f32 array into e4m3 hi + lo (returned as ml_dtypes arrays)."""
    import ml_dtypes

    e4 = ml_dtypes.float8_e4m3fn
    hi = a.astype(e4)
    lo = (a - hi.astype(np.float32)).astype(e4)
    return hi, lo


def kernel(x, Wqkv, bqkv, Wproj, bproj, _trace=False):
    x = np.asarray(x, dtype=np.float32)
    Wqkv = np.asarray(Wqkv, dtype=np.float32)
    bqkv = np.asarray(bqkv, dtype=np.float32)
    Wproj = np.asarray(Wproj, dtype=np.float32)
    bproj = np.asarray(bproj, dtype=np.float32)

    import ml_dtypes

    bf16 = ml_dtypes.bfloat16
    # [key, query] diagonal triangle: allow key <= query
    mask = np.triu(np.ones((128, 128), dtype=np.float32)).astype(bf16)
    in_maps = []
    for hg in range(2):
        sl = slice(hg * HGF, (hg + 1) * HGF)
        rows = np.concatenate(
            [Wqkv[sl], Wqkv[1024 + hg * HGF:1024 + (hg + 1) * HGF],
             Wqkv[2048 + hg * HGF:2048 + (hg + 1) * HGF]]
        )
        wqkvT = np.ascontiguousarray(rows.T) * SB          # [C, 1536], 64x
        w_hi, w_lo = _hi_lo(wqkvT)
        # q/k part (cols 0:1024): m-tile pack [1024, 1024]
        #   row = 128*m + p holds W'[256*pr + 128*two + p, 128*m + c]
        def pack_qk(wa):
            blk = wa.astype(np.float32)[:, 0:1024]
            out = np.empty((1024, 1024), dtype=np.float32)
            for m in range(8):
                b4 = blk[:, 128 * m:128 * m + 128].reshape(4, 2, 128, 128)
                out[128 * m:128 * m + 128] = (
                    b4.transpose(2, 0, 1, 3).reshape(128, 1024)
                )
            return out

        # v part (cols 1024:1536): moving pack [512, 1024]
        #   row = 128*pr + p holds W'[256*pr + 128*two + p, 1024 + c]
        def pack_v(wa):
            blk = wa.astype(np.float32)[:, 1024:1536].reshape(4, 2, 128, 512)
            return blk.transpose(0, 2, 1, 3).reshape(512, 1024)

        e4 = ml_dtypes.float8_e4m3fn
        wqkh_np = np.ascontiguousarray(pack_qk(w_hi)).astype(e4)
        wqkl_np = np.ascontiguousarray(pack_qk(w_lo)).astype(e4)
        wvh_np = np.ascontiguousarray(pack_v(w_hi)).astype(e4)
        wvl_np = np.ascontiguousarray(pack_v(w_lo)).astype(e4)
        bq = np.ascontiguousarray(
            (np.concatenate(
                [bqkv[sl], bqkv[1024 + hg * HGF:1024 + (hg + 1) * HGF]]
            ) * (SA * SB)).reshape(8, 128).T
        ).astype(np.float32)
        # proj weights [512, 1024] o-feat rows, 64x; pack [256, 2048]:
        #   row = 128*g + p holds Wp'[256*g + 128*two + p, c]
        wprojT = np.ascontiguousarray(Wproj[:, sl].T) * SB
        wp_hi, wp_lo = _hi_lo(wprojT)

        def pack_wp(wa):
            blk = wa.astype(np.float32).reshape(2, 2, 128, 1024)
            return blk.transpose(0, 2, 1, 3).reshape(256, 2048)

        wph_np = np.ascontiguousarray(pack_wp(wp_hi)).astype(e4)
        wpl_np = np.ascontiguousarray(pack_wp(wp_lo)).astype(e4)
        for b in range(B):
            # x pack: [512, 4096]: row = 128*pr + p,
            # col = 1024*n + 512*two + t holds x'[256*pr + 128*two + p, t]
            xT = np.ascontiguousarray(x[b].T) * SA         # [C, T], 16x
            x_hi, x_lo = _hi_lo(xT)

            def pack_x(xa):
                blk = xa.astype(np.float32).reshape(4, 2, 128, 4, 512)
                return blk.transpose(0, 2, 3, 1, 4).reshape(512, 4096)

            in_maps.append(
                {
                    "xh": np.ascontiguousarray(pack_x(x_hi)).astype(e4),
                    "xl": np.ascontiguousarray(pack_x(x_lo)).astype(e4),
                    "wqkh": wqkh_np,
                    "wqkl": wqkl_np,
                    "wvh": wvh_np,
                    "wvl": wvl_np,
                    "bqk": bq,
                    "wph": wph_np,
                    "wpl": wpl_np,
                    "mask01": mask,
                }
            )
    # core order: idx = hg * 4 + b  (in_maps built hg-major already)
    in_maps = in_maps[:4] + in_maps[4:]
    res = run_bass_kernel_spmd(_get_nc(), in_maps, core_ids=list(range(8)),
                               trace=_trace)
    # V-bias folds into a constant output row: softmax rows sum to 1, so
    # y += (Wproj @ bv) for the full bv (both head groups combined)
    bias_row = bproj + Wproj @ bqkv[2 * C:3 * C]
    out = np.empty((B, T, C), dtype=np.float32)
    for b in range(B):
        out[b] = (res.results[b]["y"] + res.results[4 + b]["y"]) / OUT_DIV \
            + bias_row
    if _trace:
        return out, res
    return out


# revision 64
# speedup vs baseline: 1.0231x; 1.0000x over previous
"""Causal self-attention (B=4, T=2048, C=1024, NH=16) on 8 trn2 NeuronCores.

Sharding: core = (head_group hg in {0,1}) x (batch b in {0..3}).
Each core computes qkv projection + attention + partial output projection for
its 8 heads of its batch; host sums the two head-group partials per batch and
adds the output bias.

v3 = v2 + fp8 DoubleRow "3-slot split" for the qkv and output projections.
  - A matmul instruction costs out_free_size x cycles_per_row; fp8e4/e5 with
    perf_mode=DoubleRow runs at 0.5 cycles/row and contracts TWO 128-row
    k-tiles per instruction (lhsT [128,2,M], rhs [128,2,N]).  Writing
    X = Xh + Xl and W = Wh + Wl (each an e4m3 pair: hi = e4(x), lo =
    e4(x - hi), together ~9 mantissa bits > bf16's 8), the product
    X@W ~= Xh@Wh + Xh@Wl + Xl@Wh needs 3 slot-products per k-tile = 1.5
    DoubleRow instructions per k-tile pair = 0.75x the bf16 PE cost.
    Measured end-to-end rel-err 4.3e-3 vs bf16's 5.6e-3 (the e4m3 pair is
    slightly MORE precise than bf16).
  - Scale scheme keeps every fp8 operand in e4m3 normal range with all
    compensation factors exact powers of two: x' = 16x, W' = 64Wqkv =>
    PSUM q/k/v are 1024x; bias ships 1024x; scores are 2^20 x so the exp
    scale is 0.125*2^-20; the AV ones-column is 64.0 so o = acc*dinv comes
    out 16x; Wproj' = 64Wproj => y partials are 1024x, divided on the host.
  - qkv/V weights and x ship as host-packed hi/lo e4m3 pairs in DR-friendly
    row order (slot pairs contiguous), so SBUF tiles load with plain DMAs:
    same total bytes as the bf16 v2 (hi+lo = 2 bytes/elem).
  - o (= 16x true o, bf16) transposes to oT exactly as v2 (one
    dma_start_transpose per 128-query chunk), then splits on DVE into
    oT_hi = e4(oT), oT_lo = e4(oT - oT_hi) for the deferred DR projection.
  - scores and AV stay bf16: the score contraction is only 64 (no k-tile
    pair to fuse) and AV's pt residual would need a second exp pass.

Attention core (unchanged from v2):
  - q/k computed transposed (head_size on partitions); V in natural
    [token, feat] layout; V bias folded into the host-side output bias.
  - S^T = K @ Q^T per (head, 128-key block, 128-query chunk); 8 causal
    blocks packed in one [128, 1024] PSUM tile so one ScalarE Exp covers
    them -- and for j >= PT8J BOTH heads of a pair stream through shared
    groups (pair-merged: 144 exp instructions instead of 160; acc tiles
    allocate lazily per head, each head's dinv/o-scale emits the moment
    its last AV lands so the single acc bank recycles mid-group).
    Causal mask = one 0/1 multiply per diagonal block on GpSimd.
  - AV runs query-on-partitions: acc[q, 0:65] += pt_slice^T @ [V | 64];
    the softmax denominator arrives as a per-partition scalar -> DVE
    reciprocal + tensor_scalar_mul.
  - output projections deferred to the end of the program as PE fill for
    the ACT-bound late tiles.
  - a few throwaway warmup matmuls burn the PE p-state ramp.
Cost-model span: 180580 ns/core (v2 bf16: 203588; original stub: 309000).
Engine busy: PE 160.3us, ACT 151.9us, DVE 105.2us, Pool 45.0us, SP 29.2us.
"""

import sys

sys.path.insert(0, "/opt/trn_rl_repo")

import numpy as np

import concourse.bacc as bacc
import concourse.bass as bass
import concourse.mybir as mybir
from concourse.bass_utils import run_bass_kernel_spmd
from concourse.tile import TileContext

B, T, C, NH = 4, 2048, 1024, 16
HS = C // NH          # 64
HGF = 512             # features per head group (8 heads x 64)
QT = 256              # query tile (S stage)
NKT = T // 128        # 16 key tiles
F32 = mybir.dt.float32
BF16 = mybir.dt.bfloat16
E4 = mybir.dt.float8e4
DR = mybir.MatmulPerfMode.DoubleRow
Exp = mybir.ActivationFunctionType.Exp

SA = 16.0             # x scale
SB = 64.0             # weight scale
EXP_SCALE = 0.125 / float(SA * SA * SB * SB)   # 0.125 * 2^-20
ONES_VAL = 64.0       # AV denominator column value => o = 16x true o
OUT_DIV = SA * SB     # host divides y partials by 1024
# query chunks j >= PT8J run attention weights in e4m3: exp emits pt8 =
# exp(s - PT8_SHIFT) (max scaled score is 9.04 -> pt8 <= e^5.04 = 155 < 448;
# rows have >= 513 keys so a row of all-zero pt8 is impossible), V ships as
# a 16x e4m3 hi/lo pair, and AV runs DoubleRow over key-tile pairs at half
# the bf16 PE cost.  The pt8 denominator column (1.0) normalizes with the
# same quantized weights, cancelling most of the quantization error.
PT8J = 2
PT8_SHIFT = -4.0
RG = 1                # S-groups fused per exp instruction (1 = double-buffered)


def build_kernel():
    nc = bacc.Bacc(None, target_bir_lowering=False)
    # packed x: row = 128*pr + p, col = 1024*n + 512*two + t
    xh = nc.dram_tensor("xh", (512, 4096), E4, kind="ExternalInput")
    xl = nc.dram_tensor("xl", (512, 4096), E4, kind="ExternalInput")
    # packed q/k weights: row = 128*m + p (m 0..3 q, 4..7 k),
    # col = 256*pr + 128*two + c
    wqkh = nc.dram_tensor("wqkh", (1024, 1024), E4, kind="ExternalInput")
    wqkl = nc.dram_tensor("wqkl", (1024, 1024), E4, kind="ExternalInput")
    # packed v weights (moving layout): row = 128*pr + p, col = 512*two + c
    wvh = nc.dram_tensor("wvh", (512, 1024), E4, kind="ExternalInput")
    wvl = nc.dram_tensor("wvl", (512, 1024), E4, kind="ExternalInput")
    bqk = nc.dram_tensor("bqk", (128, 8), F32, kind="ExternalInput")
    # packed proj weights: row = 128*g + p, col = 1024*two + c
    wph = nc.dram_tensor("wph", (256, 2048), E4, kind="ExternalInput")
    wpl = nc.dram_tensor("wpl", (256, 2048), E4, kind="ExternalInput")
    mask01 = nc.dram_tensor("mask01", (128, 128), BF16, kind="ExternalInput")
    y = nc.dram_tensor("y", (T, C), F32, kind="ExternalOutput")

    with TileContext(nc) as tc:
        with (
            tc.tile_pool(name="outer", bufs=1) as outer,
            tc.tile_pool(name="work", bufs=1) as work,
            tc.tile_pool(name="psum", bufs=1, space="PSUM") as psum,
        ):
            # ---- PE p-state warmup ----
            warm = outer.tile([128, 512], BF16, name="warm")
            nc.vector.memset(warm, 0.0)
            for wi in range(5):
                wtag, wbufs = ("py", 1) if wi % 2 == 0 else ("mm", 2)
                wps = psum.tile([128, 512], F32, tag=wtag, bufs=wbufs,
                                name=f"warm{wi}")
                nc.tensor.matmul(wps, warm[:, 0:128], warm,
                                 start=True, stop=True)

            bias_all = outer.tile([128, 8], F32, name="bias_all")
            mask_b = outer.tile([128, 128], BF16, name="mask_b")
            # q/k weight m-tiles [p, pr, two, c]; q part (m<4) loads first
            # on the idle scalar queue so the first qkv chain isn't blocked
            wqk_h = [outer.tile([128, 4, 2, 128], E4, name=f"wqh{m}")
                     for m in range(8)]
            wqk_l = [outer.tile([128, 4, 2, 128], E4, name=f"wql{m}")
                     for m in range(8)]
            # pair-priority order: the first S block needs q m0 AND k m4.
            # Only pairs 0/1 ride the scalar queue: its DMA issues occupy
            # the ACT sequencer ~500ns each and would otherwise queue-block
            # the first exp until ~8us.  Pairs 2/3 load later via sync /
            # gpsimd (emitted in the chunk-0 extras below).
            for m in (0, 4, 1, 5):
                nc.scalar.dma_start(
                    wqk_h[m], wqkh[128 * m:128 * m + 128, :].rearrange(
                        "p (pr two c) -> p pr two c", two=2, c=128))
                nc.scalar.dma_start(
                    wqk_l[m], wqkl[128 * m:128 * m + 128, :].rearrange(
                        "p (pr two c) -> p pr two c", two=2, c=128))
            wv_h = [outer.tile([128, 2, 512], E4, name=f"wvh{pr}")
                    for pr in range(4)]
            wv_l = [outer.tile([128, 2, 512], E4, name=f"wvl{pr}")
                    for pr in range(4)]
            wp_h = [outer.tile([128, 2, 1024], E4, name=f"wph{g}")
                    for g in range(2)]
            wp_l = [outer.tile([128, 2, 1024], E4, name=f"wpl{g}")
                    for g in range(2)]

            k_t = [outer.tile([128, T], BF16, name=f"k{i}") for i in range(4)]
            q_sb = [outer.tile([128, T], BF16, name=f"q{i}") for i in range(4)]
            # v_store[i] (bf16, j < PT8J only): [key-tile 128, 8*65]; per head
            # h cols 65h:65h+64 are V features (1024x), col 65h+64 is
            # ONES_VAL (softmax denominator)
            v_store = [outer.tile([128, 8 * 65], BF16, name=f"v{i}")
                       for i in range(2 * PT8J)]
            for i in range(2 * PT8J):
                nc.vector.memset(
                    v_store[i].rearrange("p (g c) -> p g c", c=65)[:, :, 64:65],
                    ONES_VAL,
                )
            # vp_hi/vp_lo[ip] (e4m3, j >= PT8J): key-tile PAIR (2ip, 2ip+1)
            # as DoubleRow slots; per head h cols 65h:65h+64 hold 16x V
            # hi/lo, col 65h+64 is 1.0 in hi / 0.0 in lo (denominator)
            vp_hi = [outer.tile([128, 2, 8 * 65], E4, name=f"vph{ip}")
                     for ip in range(NKT // 2)]
            vp_lo = [outer.tile([128, 2, 8 * 65], E4, name=f"vpl{ip}")
                     for ip in range(NKT // 2)]
            for ip in range(NKT // 2):
                nc.vector.memset(
                    vp_hi[ip].rearrange("p t (g c) -> p t g c", c=65)[:, :, :, 64:65],
                    1.0,
                )
                nc.vector.memset(
                    vp_lo[ip].rearrange("p t (g c) -> p t g c", c=65)[:, :, :, 64:65],
                    0.0,
                )
            nb_sh = outer.tile([128, 1], F32, name="nb_sh")
            nc.vector.memset(nb_sh, PT8_SHIFT)

            # ---- deferred/interleaved output projection machinery ----
            # ready_proj holds (j, s, oT_hi, oT_lo, nn) psy groups whose oT
            # splits are already emitted; emit_psy pops them into the PE
            # stream.  Interleaved groups drain on DVE (ACT is exp-saturated
            # in the late chunks); tail groups split drains as in v2.
            ready_proj = []
            psy_count = [0]

            def emit_psy(j, s, oT_hi, oT_lo, nn, tag, bufs, drain):
                psy = psum.tile([128, 512], F32, tag=tag, bufs=bufs,
                                name=f"py{j}_{s}_{nn}")
                kk = 0
                for g in range(2):
                    for (lt, rt) in ((oT_hi, wp_h[g]), (oT_lo, wp_h[g]),
                                     (oT_hi, wp_l[g])):
                        nc.tensor.matmul(
                            psy,
                            lt[:, 2 * g:2 * g + 2, :],
                            rt[:, :, nn * 512:(nn + 1) * 512],
                            start=(kk == 0), stop=(kk == 5),
                            perf_mode=DR,
                        )
                        kk += 1
                ysb = work.tile([128, 512], F32, tag="ysb", bufs=6,
                                name=f"ys{j}_{s}_{nn}")
                rows = y[j * QT + s * 128:j * QT + (s + 1) * 128, :]
                if drain == "final":
                    # final drain: halves in parallel on two engines and
                    # four DMA queues to shorten the closing chain
                    nc.scalar.activation(
                        ysb[:, 0:256], psy[:, 0:256],
                        mybir.ActivationFunctionType.Copy,
                    )
                    nc.vector.tensor_copy(ysb[:, 256:512], psy[:, 256:512])
                    q0, q1 = ((nc.sync, nc.gpsimd) if psy_count[0] % 2 == 0
                              else (nc.scalar, nc.sync))
                    q0.dma_start(
                        rows[:, nn * 512:nn * 512 + 256], ysb[:, 0:256]
                    )
                    q1.dma_start(
                        rows[:, nn * 512 + 256:nn * 512 + 512],
                        ysb[:, 256:512],
                    )
                else:
                    if drain == "act":
                        nc.scalar.activation(
                            ysb, psy, mybir.ActivationFunctionType.Copy
                        )
                    else:
                        nc.vector.tensor_copy(ysb, psy)
                    yeng = nc.sync if psy_count[0] % 2 == 0 else nc.gpsimd
                    yeng.dma_start(rows[:, nn * 512:(nn + 1) * 512], ysb)
                psy_count[0] += 1

            # pend_av carries the last S-group of a block whose AV matmuls
            # (exp-dependent) haven't been emitted yet; flush_av emits them
            # plus the block's dinv / o-scale ops.
            pend_av = [None]
            # filler machinery: one deferred-proj psy group per attention
            # round gives the in-order PE exp-independent work while ACT
            # drains the single-buffered S tile
            fill_state = {"budget": 0, "rate": 1, "tick": 0}
            fillers = []

            def fill_one():
                if fillers:
                    fillers.pop(0)()
                    return
                fill_state["tick"] += 1
                if (fill_state["budget"] > 0 and ready_proj
                        and fill_state["tick"] % fill_state["rate"] == 0):
                    fill_state["budget"] -= 1
                    emit_psy(*ready_proj.pop(0), tag="py", bufs=1,
                             drain="dve")

            def emit_av(pt_, grp_, acc_, j_, h_):
                for bi, (s, i) in enumerate(grp_):
                    nc.tensor.matmul(
                        acc_[s],
                        pt_[:, bi * 128:(bi + 1) * 128],
                        v_store[i][:, 65 * h_:65 * h_ + 65],
                        start=(i == 0),
                        stop=(i == 2 * j_ + s),
                    )

            post_flush = [None]

            def flush_av():
                if pend_av[0] is None:
                    return
                fn = pend_av[0]
                pend_av[0] = None
                fn()
                if post_flush[0] is not None:
                    fn2 = post_flush[0]
                    post_flush[0] = None
                    fn2()

            oT_parts = {}

            def emit_oT_half(osb, j, s, b0, b1, eng=None):
                # transpose + split feature blocks [b0, b1) (128 feats each,
                # 2 heads per block) of o tile (j, s); pieces of the last
                # chunk's tiles go out as soon as their heads complete,
                # shortening the closing chain
                if (j, s) not in oT_parts:
                    oT_parts[(j, s)] = (
                        work.tile([128, 4, 128], BF16, tag="ot", bufs=4,
                                  name=f"ot{j}_{s}"),
                        work.tile([128, 4, 128], E4, tag="oth", bufs=16,
                                  name=f"oth{j}_{s}"),
                        work.tile([128, 4, 128], E4, tag="otl", bufs=16,
                                  name=f"otl{j}_{s}"),
                    )
                oT, oT_hi, oT_lo = oT_parts[(j, s)]
                eng = eng or nc.vector
                nc.sync.dma_start_transpose(
                    oT[:, b0:b1, :], osb[j][s][:, 128 * b0:128 * b1])
                eng.tensor_copy(oT_hi[:, b0:b1, :], oT[:, b0:b1, :])
                eng.tensor_tensor(
                    oT_lo[:, b0:b1, :], oT[:, b0:b1, :], oT_hi[:, b0:b1, :],
                    mybir.AluOpType.subtract)

            def emit_oT_split(osb, j, s):
                oT = work.tile([128, 4, 128], BF16, tag="ot", bufs=4,
                               name=f"ot{j}_{s}")
                nc.sync.dma_start_transpose(oT, osb[j][s])
                oT_hi = work.tile([128, 4, 128], E4, tag="oth",
                                  bufs=16, name=f"oth{j}_{s}")
                oT_lo = work.tile([128, 4, 128], E4, tag="otl",
                                  bufs=16, name=f"otl{j}_{s}")
                nc.vector.tensor_copy(oT_hi, oT)
                nc.vector.tensor_tensor(
                    oT_lo, oT, oT_hi, mybir.AluOpType.subtract)
                for nn in range(2):
                    ready_proj.append((j, s, oT_hi, oT_lo, nn))

            def emit_attn_pair8(j, p, osb):
                """Pair-merged attention for j >= PT8J: BOTH heads of pair p
                stream through shared S-groups so one exp instruction covers
                up to 8 blocks regardless of the head boundary (160 -> 144
                exps).  acc tiles allocate lazily per head inside the AV
                stream; a head's dinv/o-scale emit the moment its last AV
                lands, releasing the single acc bank for the next head.
                """
                pair = p
                units = []   # (h, s, a, b); b None = s=0 diagonal single
                for h in (2 * p, 2 * p + 1):
                    for a in range(0, 2 * j, 2):
                        units.append((h, 0, a, a + 1))
                    units.append((h, 0, 2 * j, None))
                    for a in range(0, 2 * j + 2, 2):
                        units.append((h, 1, a, a + 1))
                groups, cur, cnt = [], [], 0
                for u in units:
                    w = 1 if u[3] is None else 2
                    if cnt + w > 8:
                        groups.append(cur)
                        cur, cnt = [], 0
                    cur.append((u, cnt))
                    cnt += w
                if cur:
                    groups.append(cur)
                accs = {}

                def av8(pt_, gu, base):
                    for (h, sx, a, b), ca in gu:
                        if h not in accs:
                            acc2 = psum.tile([128, 512], F32, tag="acc",
                                             bufs=1, name=f"acc{j}_{h}")
                            accs[h] = [acc2[:, 256 * s2:256 * s2 + 65]
                                       for s2 in range(2)]
                        acc_ = accs[h]
                        c0 = base + ca * 128
                        st = (a == 0)
                        if b is None:  # s=0 diagonal single: plain fp8 x2
                            for vt, last in ((vp_hi, False), (vp_lo, True)):
                                nc.tensor.matmul(
                                    acc_[sx],
                                    pt_[:, c0:c0 + 128],
                                    vt[a // 2][:, a % 2, 65 * h:65 * h + 65],
                                    start=False,
                                    stop=(sx == 0 and last),
                                )
                        else:
                            ptp = pt_[:, c0:c0 + 256].rearrange(
                                "p (two c) -> p two c", two=2)
                            for vt, last in ((vp_hi, False), (vp_lo, True)):
                                nc.tensor.matmul(
                                    acc_[sx], ptp,
                                    vt[a // 2][:, :, 65 * h:65 * h + 65],
                                    start=(st and vt is vp_hi),
                                    stop=(sx == 1 and b == 2 * j + 1
                                          and last),
                                    perf_mode=DR,
                                )
                        if sx == 1 and b == 2 * j + 1:
                            # head h complete: normalize + release its acc
                            for s2 in range(2):
                                dinv = work.tile([128, 1], F32, tag="dinv",
                                                 bufs=4,
                                                 name=f"di{j}_{h}_{s2}")
                                nc.vector.reciprocal(dinv, acc_[s2][:, 64:65])
                                nc.vector.tensor_scalar_mul(
                                    osb[j][s2][:, 64 * h:64 * h + 64],
                                    acc_[s2][:, 0:64],
                                    dinv,
                                )
                            del accs[h]

                prev_rnd = None
                for r, gu in enumerate(groups):
                    ntot = sum(1 if u[3] is None else 2 for u, _ in gu) * 128
                    sg = psum.tile([128, 1024], F32, tag="big", bufs=2,
                                   name=f"sg{j}_{p}_{r}")
                    for (h, sx, a, b), ca in gu:
                        off = 64 * (h % 2)
                        for bi, i in enumerate([a] if b is None else [a, b]):
                            c0 = (ca + bi) * 128
                            nc.tensor.matmul(
                                sg[:, c0:c0 + 128],
                                k_t[pair][off:off + 64,
                                          i * 128:(i + 1) * 128],
                                q_sb[pair][off:off + 64,
                                           j * QT + sx * 128:
                                           j * QT + sx * 128 + 128],
                                start=True,
                                stop=True,
                            )
                    pt = work.tile([128, 1024], E4, tag="p8", bufs=6,
                                   name=f"p8{j}_{p}_{r}")
                    nc.scalar.activation(
                        pt[:, :ntot], sg[:, :ntot],
                        Exp, scale=EXP_SCALE, bias=nb_sh,
                    )
                    for (h, sx, a, b), ca in gu:
                        if b is None and a == 2 * j:
                            c0 = ca * 128
                        elif b == 2 * j + 1 and sx == 1:
                            c0 = (ca + 1) * 128
                        else:
                            continue
                        nc.gpsimd.tensor_mul(
                            pt[:, c0:c0 + 128], pt[:, c0:c0 + 128],
                            mask_b,
                        )
                    if r == 0:
                        fill_one()
                        flush_av()
                    if prev_rnd is not None:
                        av8(prev_rnd[0], prev_rnd[1], 0)
                    if r > 0:
                        fill_one()
                    prev_rnd = (pt, gu)
                pend_av[0] = (lambda pt_=prev_rnd[0], gu_=prev_rnd[1]:
                              av8(pt_, gu_, 0))

            x_tiles = {}

            def emit_x_dma(c):
                x_h, x_l = [], []
                for pr in range(4):
                    xt_h = work.tile([128, 2, 512], E4, tag=f"xh{pr}", bufs=3,
                                     name=f"xh{c}_{pr}")
                    xt_l = work.tile([128, 2, 512], E4, tag=f"xl{pr}", bufs=3,
                                     name=f"xl{c}_{pr}")
                    # chunk 0 split across two queues so all 8 tiles beat the
                    # first qkv accumulation chain
                    heng = nc.sync if (c == 0 and pr >= 2) else nc.gpsimd
                    leng = nc.sync if (c == 0 and pr < 2) else nc.gpsimd
                    heng.dma_start(
                        xt_h, xh[128 * pr:128 * pr + 128,
                                 1024 * c:1024 * c + 1024].rearrange(
                            "p (two c) -> p two c", two=2))
                    leng.dma_start(
                        xt_l, xl[128 * pr:128 * pr + 128,
                                 1024 * c:1024 * c + 1024].rearrange(
                            "p (two c) -> p two c", two=2))
                    x_h.append(xt_h)
                    x_l.append(xt_l)
                x_tiles[c] = (x_h, x_l)

            chains_done = set()

            def emit_chain(c, m):
                if ("qk", c, m) in chains_done:
                    return
                chains_done.add(("qk", c, m))
                x_h, x_l = x_tiles[c]
                ps = psum.tile([128, 512], F32, tag="mm", bufs=2,
                               name=f"ps{c}_{m}")
                kk = 0
                for pr in range(4):
                    for (wt, xt) in ((wqk_h[m], x_h[pr]),
                                     (wqk_l[m], x_h[pr]),
                                     (wqk_h[m], x_l[pr])):
                        nc.tensor.matmul(
                            ps, wt[:, pr], xt,
                            start=(kk == 0), stop=(kk == 11),
                            perf_mode=DR,
                        )
                        kk += 1
                dst = q_sb[m] if m < 4 else k_t[m - 4]
                nc.vector.tensor_scalar_add(
                    dst[:, c * 512:(c + 1) * 512], ps,
                    bias_all[:, m:m + 1]
                )

            def emit_vchain(c, t4):
                if ("v", c, t4) in chains_done:
                    return
                chains_done.add(("v", c, t4))
                x_h, x_l = x_tiles[c]
                vtag, vbufs = (("acc", 1) if c == 0 and t4 == 2
                               else ("mm", 2))
                ps = psum.tile([128, 512], F32, tag=vtag,
                               bufs=vbufs, name=f"psv{c}_{t4}")
                kk = 0
                for pr in range(4):
                    xs_h = x_h[pr][:, :, t4 * 128:(t4 + 1) * 128]
                    xs_l = x_l[pr][:, :, t4 * 128:(t4 + 1) * 128]
                    for (lt, rt) in ((xs_h, wv_h[pr]),
                                     (xs_h, wv_l[pr]),
                                     (xs_l, wv_h[pr])):
                        nc.tensor.matmul(
                            ps, lt, rt,
                            start=(kk == 0), stop=(kk == 11),
                            perf_mode=DR,
                        )
                        kk += 1
                idx = 4 * c + t4
                if idx < 2 * PT8J:
                    nc.vector.tensor_copy(
                        v_store[idx].rearrange(
                            "p (g c) -> p g c", c=65)[:, :, 0:64],
                        ps.rearrange("p (g c) -> p g c", c=64),
                    )
                ip, sl = idx // 2, idx % 2
                hi_sl = vp_hi[ip][:, sl].rearrange(
                    "p (g c) -> p g c", c=65)[:, :, 0:64]
                lo_sl = vp_lo[ip][:, sl].rearrange(
                    "p (g c) -> p g c", c=65)[:, :, 0:64]
                ps_v = ps.rearrange("p (g c) -> p g c", c=64)
                # hi = e4(ps/64) = e4(16 v); lo = e4(ps/64 - hi)
                nc.vector.tensor_scalar_mul(hi_sl, ps_v, 1.0 / 64.0)
                nc.vector.scalar_tensor_tensor(
                    lo_sl, ps_v, 1.0 / 64.0, hi_sl,
                    mybir.AluOpType.mult, mybir.AluOpType.subtract,
                )

            # ---- diagonal-major unit schedule ----
            # A unit (c, p) = qkv chains for head-pair p of chunk c + that
            # pair's attention blocks.  The exp-rich pair-0 units of chunks
            # 2/3 are pulled EARLY (into the PE-bound phases of chunks 1/2)
            # so the ACT exp stream never runs dry during the causal ramp.
            UNITS = [(c, p) for c in range(4) for p in range(4)]
            # x DMAs issue well before each chunk's first unit
            X_PREFETCH = {(0, 0): [0], (0, 2): [1], (1, 1): [2], (2, 1): [3]}
            PSY_BUDGET = {}
            o_sbs = {}
            for ui, (c, p) in enumerate(UNITS):
                for cpre in X_PREFETCH.get((c, p), []):
                    emit_x_dma(cpre)
                    if cpre == 0:
                        # late-needed loads behind the first x chunk;
                        # bias/mask first (needed by the first bias-add)
                        nc.gpsimd.dma_start(bias_all, bqk[:, :])
                        nc.gpsimd.dma_start(mask_b, mask01[:, :])
                        for pr in range(4):
                            veng = nc.gpsimd if pr < 2 else nc.sync
                            veng.dma_start(
                                wv_h[pr],
                                wvh[128 * pr:128 * pr + 128, :].rearrange(
                                    "p (two c) -> p two c", two=2))
                            veng.dma_start(
                                wv_l[pr],
                                wvl[128 * pr:128 * pr + 128, :].rearrange(
                                    "p (two c) -> p two c", two=2))
                        for m in (2, 6):
                            nc.sync.dma_start(
                                wqk_h[m],
                                wqkh[128 * m:128 * m + 128, :].rearrange(
                                    "p (pr two c) -> p pr two c", two=2,
                                    c=128))
                            nc.sync.dma_start(
                                wqk_l[m],
                                wqkl[128 * m:128 * m + 128, :].rearrange(
                                    "p (pr two c) -> p pr two c", two=2,
                                    c=128))
                        for m in (3, 7):
                            nc.gpsimd.dma_start(
                                wqk_h[m],
                                wqkh[128 * m:128 * m + 128, :].rearrange(
                                    "p (pr two c) -> p pr two c", two=2,
                                    c=128))
                            nc.gpsimd.dma_start(
                                wqk_l[m],
                                wqkl[128 * m:128 * m + 128, :].rearrange(
                                    "p (pr two c) -> p pr two c", two=2,
                                    c=128))
                        for g in range(2):
                            nc.sync.dma_start(
                                wp_h[g],
                                wph[128 * g:128 * g + 128, :].rearrange(
                                    "p (two c) -> p two c", two=2))
                            nc.sync.dma_start(
                                wp_l[g],
                                wpl[128 * g:128 * g + 128, :].rearrange(
                                    "p (two c) -> p two c", two=2))
                if p == 0:
                    o_sbs[c] = {
                        j: [
                            work.tile([128, HGF], BF16, tag=f"os{s}", bufs=4,
                                      name=f"o{j}_{s}")
                            for s in range(2)
                        ]
                        for j in (2 * c, 2 * c + 1)
                    }
                o_sb = o_sbs[c]
                if (c, p) in PSY_BUDGET:
                    fill_state.update(budget=PSY_BUDGET[(c, p)], tick=0,
                                      rate=(2 if c < 3 else 1))
                if c == 3 and p == 2:
                    # j=6 heads 0-3 (cols 0:256) complete once (6,3)'s
                    # pending AV flushes: transpose/split that half early
                    post_flush[0] = lambda osb_=o_sb: (
                        emit_oT_half(osb_, 6, 0, 0, 2),
                        emit_oT_half(osb_, 6, 1, 0, 2))
                if c == 3 and p == 3:
                    # heads 4,5 (block 2) complete once (6,5) flushes
                    post_flush[0] = lambda osb_=o_sb: (
                        emit_oT_half(osb_, 6, 0, 2, 3),
                        emit_oT_half(osb_, 6, 1, 2, 3))
                for m in (p, 4 + p):  # q then k, transposed layout
                    emit_chain(c, m)
                if p == 0:
                    # V tiles 0,1 now; 2,3 ride between the j=2c and j=2c+1
                    # blocks (chunk 3 runs j=7 first and needs all four)
                    emit_vchain(c, 0)
                    emit_vchain(c, 1)
                    if c == 3:
                        emit_vchain(c, 2)
                        emit_vchain(c, 3)
                # chunk 3 runs j=7 before j=6 so j=7's o tiles finish
                # (and start their projections) while j=6 is still in
                # flight, shortening the closing chain
                if c == 3:
                    jh_list = [(7, 2 * p), (7, 2 * p + 1),
                               (6, 2 * p), (6, 2 * p + 1)]
                else:
                    jh_list = [(2 * c, 2 * p), (2 * c, 2 * p + 1),
                               (2 * c + 1, 2 * p), (2 * c + 1, 2 * p + 1)]
                for bi_jh, (j, h) in enumerate(jh_list):
                    if p == 0 and bi_jh == 2 and c < 3:
                        emit_vchain(c, 2)
                        emit_vchain(c, 3)
                    pair, off = h // 2, 64 * (h % 2)
                        if j >= PT8J:
                            emit_attn_block8(j, h, pair, off, o_sb)
                            if n == 3 and p == 3 and j == 7 and h == 7:
                                flush_av()
                                emit_oT_split(7, 0)
                                emit_oT_split(7, 1)
                            continue
                        acc2 = psum.tile([128, 512], F32, tag="acc", bufs=1,
                                         name=f"acc{j}_{h}")
                        acc = [acc2[:, 256 * s:256 * s + 65] for s in range(2)]
                        blocks = [(s, i) for s in range(2)
                                  for i in range(2 * j + s + 1)]
                        # software pipeline: each group's AV matmuls are
                        # emitted after the NEXT group's S matmuls (the AV
                        # waits on this group's exp; the next S does not), so
                        # the in-order PE always has exp-independent work
                        # while ACT exponentiates.  The last group's AV is
                        # carried into the next block via pend_av.
                        prev_grp = None
                        for g in range((len(blocks) + 7) // 8):
                            grp = blocks[8 * g:8 * g + 8]
                            sg = psum.tile([128, 1024 * RG], F32, tag="big",
                                           bufs=(2 if RG == 1 else 1),
                                           name=f"sg{j}_{h}_{g}")
                            for bi, (s, i) in enumerate(grp):
                                nc.tensor.matmul(
                                    sg[:, bi * 128:(bi + 1) * 128],
                                    k_t[pair][off:off + 64, i * 128:(i + 1) * 128],
                                    q_sb[pair][off:off + 64,
                                               j * QT + s * 128:
                                               j * QT + s * 128 + 128],
                                    start=True,
                                    stop=True,
                                )
                            pt = work.tile([128, 1024 * RG], BF16, tag="pt",
                                           bufs=2, name=f"pt{j}_{h}_{g}")
                            nc.scalar.activation(
                                pt[:, :len(grp) * 128], sg[:, :len(grp) * 128],
                                Exp, scale=EXP_SCALE
                            )
                            for bi, (s, i) in enumerate(grp):
                                if i == 2 * j + s:  # diagonal triangle
                                    nc.gpsimd.tensor_mul(
                                        pt[:, bi * 128:(bi + 1) * 128],
                                        pt[:, bi * 128:(bi + 1) * 128],
                                        mask_b,
                                    )
                            if g == 0:
                                flush_av()  # previous block's last AV + o
                            if prev_grp is not None:
                                emit_av(prev_grp[0], prev_grp[1], acc, j, h)
                            prev_grp = (pt, grp)
                        def _old_flush(pt_=prev_grp[0], grp_=prev_grp[1],
                                       acc_=acc, j_=j, h_=h, osb_=o_sb):
                            emit_av(pt_, grp_, acc_, j_, h_)
                            for s2 in range(2):
                                dinv = work.tile([128, 1], F32, tag="dinv",
                                                 bufs=4,
                                                 name=f"di{j_}_{h_}_{s2}")
                                nc.vector.reciprocal(dinv, acc_[s2][:, 64:65])
                                nc.vector.tensor_scalar_mul(
                                    osb_[j_][s2][:, 64 * h_:64 * h_ + 64],
                                    acc_[s2][:, 0:64],
                                    dinv,
                                )
                        pend_av[0] = _old_flush
                # transpose + hi/lo split for this chunk's finished o tiles
                flush_av()
                if n < 3:
                    for j in (2 * n, 2 * n + 1):
                        for s in range(2):
                            emit_oT_split(j, s)
                else:
                    for s in range(2):
                        emit_oT_half(6, s, 1)
                        _, oT_hi, oT_lo = oT_parts[(6, s)]
                        for nn in range(2):
                            ready_proj.append((6, s, oT_hi, oT_lo, nn))

            # ---- tail: remaining proj groups (j=6) on four distinct PSUM
            # banks so the chains run back-to-back and drains parallelize ----
            n_tail = len(ready_proj)
            tail_tags = [("py", 1), ("mm", 2), ("mm", 2), ("acc", 1)]
            for i, item in enumerate(ready_proj):
                tag, bufs = tail_tags[i % 4]
                # early tail psys drain on DVE: their ACT copies would queue
                # behind the still-running exp stream and pin the PSUM banks
                drain = ("final" if i >= n_tail - 2 else
                         ("dve" if i < n_tail - 4 else
                          ("act" if i % 2 == 0 else "dve")))
                emit_psy(*item, tag=tag, bufs=bufs, drain=drain)

    nc.finalize()
    return nc


_NC = None


def _get_nc():
    global _NC
    if _NC is None:
        _NC = build_kernel()
    return _NC


def _hi_lo(a):
    """Split f32 array into e4m3 hi + lo (returned as ml_dtypes arrays)."""
    import ml_dtypes

    e4 = ml_dtypes.float8_e4m3fn
    hi = a.astype(e4)
    lo = (a - hi.astype(np.float32)).astype(e4)
    return hi, lo


def kernel(x, Wqkv, bqkv, Wproj, bproj, _trace=False):
    x = np.asarray(x, dtype=np.float32)
    Wqkv = np.asarray(Wqkv, dtype=np.float32)
    bqkv = np.asarray(bqkv, dtype=np.float32)
    Wproj = np.asarray(Wproj, dtype=np.float32)
    bproj = np.asarray(bproj, dtype=np.float32)

    import ml_dtypes

    bf16 = ml_dtypes.bfloat16
    # [key, query] diagonal triangle: allow key <= query
    mask = np.triu(np.ones((128, 128), dtype=np.float32)).astype(bf16)
    in_maps = []
    for hg in range(2):
        sl = slice(hg * HGF, (hg + 1) * HGF)
        rows = np.concatenate(
            [Wqkv[sl], Wqkv[1024 + hg * HGF:1024 + (hg + 1) * HGF],
             Wqkv[2048 + hg * HGF:2048 + (hg + 1) * HGF]]
        )
        wqkvT = np.ascontiguousarray(rows.T) * SB          # [C, 1536], 64x
        w_hi, w_lo = _hi_lo(wqkvT)
        # q/k part (cols 0:1024): m-tile pack [1024, 1024]
        #   row = 128*m + p holds W'[256*pr + 128*two + p, 128*m + c]
        def pack_qk(wa):
            blk = wa.astype(np.float32)[:, 0:1024]
            out = np.empty((1024, 1024), dtype=np.float32)
            for m in range(8):
                b4 = blk[:, 128 * m:128 * m + 128].reshape(4, 2, 128, 128)
                out[128 * m:128 * m + 128] = (
                    b4.transpose(2, 0, 1, 3).reshape(128, 1024)
                )
            return out

        # v part (cols 1024:1536): moving pack [512, 1024]
        #   row = 128*pr + p holds W'[256*pr + 128*two + p, 1024 + c]
        def pack_v(wa):
            blk = wa.astype(np.float32)[:, 1024:1536].reshape(4, 2, 128, 512)
            return blk.transpose(0, 2, 1, 3).reshape(512, 1024)

        e4 = ml_dtypes.float8_e4m3fn
        wqkh_np = np.ascontiguousarray(pack_qk(w_hi)).astype(e4)
        wqkl_np = np.ascontiguousarray(pack_qk(w_lo)).astype(e4)
        wvh_np = np.ascontiguousarray(pack_v(w_hi)).astype(e4)
        wvl_np = np.ascontiguousarray(pack_v(w_lo)).astype(e4)
        bq = np.ascontiguousarray(
            (np.concatenate(
                [bqkv[sl], bqkv[1024 + hg * HGF:1024 + (hg + 1) * HGF]]
            ) * (SA * SB)).reshape(8, 128).T
        ).astype(np.float32)
        # proj weights [512, 1024] o-feat rows, 64x; pack [256, 2048]:
        #   row = 128*g + p holds Wp'[256*g + 128*two + p, c]
        wprojT = np.ascontiguousarray(Wproj[:, sl].T) * SB
        wp_hi, wp_lo = _hi_lo(wprojT)

        def pack_wp(wa):
            blk = wa.astype(np.float32).reshape(2, 2, 128, 1024)
            return blk.transpose(0, 2, 1, 3).reshape(256, 2048)

        wph_np = np.ascontiguousarray(pack_wp(wp_hi)).astype(e4)
        wpl_np = np.ascontiguousarray(pack_wp(wp_lo)).astype(e4)
        for b in range(B):
            # x pack: [512, 4096]: row = 128*pr + p,
            # col = 1024*n + 512*two + t holds x'[256*pr + 128*two + p, t]
            xT = np.ascontiguousarray(x[b].T) * SA         # [C, T], 16x
            x_hi, x_lo = _hi_lo(xT)

            def pack_x(xa):
                blk = xa.astype(np.float32).reshape(4, 2, 128, 4, 512)
                return blk.transpose(0, 2, 3, 1, 4).reshape(512, 4096)

            in_maps.append(
                {
                    "xh": np.ascontiguousarray(pack_x(x_hi)).astype(e4),
                    "xl": np.ascontiguousarray(pack_x(x_lo)).astype(e4),
                    "wqkh": wqkh_np,
                    "wqkl": wqkl_np,
                    "wvh": wvh_np,
                    "wvl": wvl_np,
                    "bqk": bq,
                    "wph": wph_np,
                    "wpl": wpl_np,
                    "mask01": mask,
                }
            )
    # core order: idx = hg * 4 + b  (in_maps built hg-major already)
    in_maps = in_maps[:4] + in_maps[4:]
    res = run_bass_kernel_spmd(_get_nc(), in_maps, core_ids=list(range(8)),
                               trace=_trace)
    # V-bias folds into a constant output row: softmax rows sum to 1, so
    # y += (Wproj @ bv) for the full bv (both head groups combined)
    bias_row = bproj + Wproj @ bqkv[2 * C:3 * C]
    out = np.empty((B, T, C), dtype=np.float32)
    for b in range(B):
        out[b] = (res.results[b]["y"] + res.results[4 + b]["y"]) / OUT_DIV \
            + bias_row
    if _trace:
        return out, res
    return out
